# revision 1
# baseline (speedup 1.0000x reference)
"""Causal self-attention (B=2, T=2048, C=1024, H=16, D=64) on 8 trn2 cores.

Sharding: core c -> batch b = c // 4, head-group g = c % 4 (4 heads each).
Data-parallel over B, tensor-parallel (Megatron) over heads for the
qkv / proj linears. Each core computes its head-group's attention and a
partial output projection; the host sums the 4 partials per batch and
adds the proj bias.

Everything on-device is done in transposed [feature, token] space:
  qk^T = Wqk @ x^T                       (PE; bias added by ACT on evacuate)
  v    = x @ Wv^T (+ ones column)        (PE; K=1 matmul adds bias + ones)
  att^T[k, q] = k^T(head)^T . q(head)    (PE, K=64; causal tiles skipped)
  P = exp(att^T + additive causal mask)  (ACT; no max-subtraction needed,
                                          |logits| <~ 10 so fp32 exp is safe)
  rawout^T[d+1, q] = v_aug^T @ P         (PE accumulate over k chunks; the
                                          ones column makes row 64 = sum_k P
                                          = softmax denominator, for free)
  out^T = rawout^T[:64] * (1/denom)      (DVE recip + PE K=1 broadcast + DVE)
  y^T(partial) = Wp_g^T.T @ out^T        (PE)
"""

import os
import sys
import types

for _p in ("/opt/trn_rl_repo", "/root/.axon_site", "/root/.axon_site/_ro/trn_rl_repo"):
    if os.path.isdir(_p) and _p not in sys.path:
        sys.path.append(_p)

import numpy as np

import concourse.bacc as bacc
import concourse.bass as bass
import concourse.mybir as mybir
import concourse.tile as tile
from concourse.bass_utils import run_bass_kernel_spmd

# ── problem constants (hardcoded; spec.json not available at grade time) ──
B, T, C = 2, 2048, 1024
H, D = 16, 64
N_CORES = 8
HPG = 4                 # heads per group (per core)
CG = HPG * D            # 256 channels per head-group
NT = T // 512           # 4 token chunks of 512
KC = C // 128           # 8 contraction tiles for C
NEG = -1.0e4            # (unused) additive mask value
VW = HPG * 65 + 64      # v tile width: 4x(64+ones col) + 64 zero-pad cols

F32 = mybir.dt.float32
F32R = mybir.dt.float32r
# per-stage matmul operand dtype: float32 (exact, 4 cyc/row) or float32r
# (tf32-like, 1 cyc/row at N>=256). Overridable for A/B testing.
MMDT = {
    "qk": F32R, "v": F32R, "att": F32R, "av": F32R, "proj": F32R, "k1": F32,
}
_trace_flag = [False]   # test.py can flip this to capture a profile
_last_results = [None]


def _mm(nc, out, lhsT, rhs, stage, **kw):
    nc.tensor.matmul(out, lhsT, rhs, **kw)


def _ensure_ntff_hook():
    """Install the NTFF profile hook shim (container's antenv lacks it)."""
    if "antenv.axon_hooks" in sys.modules:
        return
    try:
        from trn_agent_boot.trn_boot import _ntff_profile_via_ctypes
    except Exception:
        return
    mod = types.ModuleType("antenv.axon_hooks")
    hook = [None]
    mod.set_axon_ntff_profile_hook = lambda h: hook.__setitem__(0, h)
    mod.get_axon_ntff_profile_hook = lambda: hook[0]
    sys.modules["antenv.axon_hooks"] = mod
    so = "/opt/axon/libaxon_pjrt.so"
    if os.path.exists(so):
        mod.set_axon_ntff_profile_hook(_ntff_profile_via_ctypes(so))


def build_nc():
    nc = bacc.Bacc("TRN2", target_bir_lowering=False, debug=False,
                   num_devices=N_CORES)

    xt_d = nc.dram_tensor("xt", [C, T], F32, kind="ExternalInput").ap()
    wqk_d = nc.dram_tensor("wqk", [C, 2 * CG], F32, kind="ExternalInput").ap()
    bqk_d = nc.dram_tensor("bqk", [2 * CG, 1], F32, kind="ExternalInput").ap()
    wv_d = nc.dram_tensor("wv", [C, VW], F32, kind="ExternalInput").ap()
    bv_d = nc.dram_tensor("bv", [1, VW], F32, kind="ExternalInput").ap()
    wp_d = nc.dram_tensor("wp", [CG, C], F32, kind="ExternalInput").ap()
    mask_d = nc.dram_tensor("mask", [128, 128], F32, kind="ExternalInput").ap()
    ones_d = nc.dram_tensor("ones", [1, 128], F32, kind="ExternalInput").ap()
    zeros_d = nc.dram_tensor("zeros", [64, T], F32, kind="ExternalInput").ap()
    yt_d = nc.dram_tensor("yt", [C, T], F32, kind="ExternalOutput").ap()
    rec_d = nc.dram_tensor("rec_scratch", [HPG * NT, 512], F32).ap()
    den_d = nc.dram_tensor("den_scratch", [HPG * NT, 512], F32).ap()

    with tile.TileContext(nc) as tc:
        with tc.tile_pool(name="const", bufs=1) as cp:
            # ── persistent SBUF residents ──
            assert MMDT["v"] == MMDT["qk"]
            xtp = tc.tile_pool(name="xtp", bufs=1)
            xtpool = xtp.__enter__()
            xt = [xtpool.tile([128, T], MMDT["qk"], tag=f"xt{k}", name=f"xt{k}") for k in range(KC)]
            wqk = [cp.tile([128, 2 * CG], MMDT["qk"], tag=f"wqk{k}", name=f"wqk{k}") for k in range(KC)]
            wv = [cp.tile([128, VW], MMDT["v"], tag=f"wv{k}", name=f"wv{k}") for k in range(KC)]
            bqk = [cp.tile([128, 1], F32, tag=f"bqk{m}", name=f"bqk{m}") for m in range(4)]
            bv = cp.tile([1, VW], MMDT["v"], tag="bv")
            wp = [cp.tile([128, C], MMDT["proj"], tag=f"wp{k}", name=f"wp{k}") for k in range(2)]
            tri = cp.tile([128, 128], MMDT["av"], tag="tri", name="tri")
            ones = cp.tile([1, 128], MMDT["v"], tag="ones")
            qk = [cp.tile([128, T], MMDT["att"], tag=f"qk{m}", name=f"qk{m}") for m in range(4)]
            v_sb = [cp.tile([128, VW], MMDT["av"], tag=f"v{m}", name=f"v{m}") for m in range(T // 128)]
            outT = [cp.tile([128, T], MMDT["proj"], tag=f"outT{k}", name=f"outT{k}") for k in range(2)]

            for k in range(KC):
                nc.sync.dma_start(wqk[k][:], wqk_d[128 * k:128 * (k + 1), :].bitcast(MMDT['qk']))
            for k in range(KC):
                nc.sync.dma_start(xt[k][:], xt_d[128 * k:128 * (k + 1), :].bitcast(MMDT['qk']))
            for k in range(KC):
                nc.sync.dma_start(wv[k][:], wv_d[128 * k:128 * (k + 1), :].bitcast(MMDT['v']))
            for m in range(4):
                nc.sync.dma_start(bqk[m][:], bqk_d[128 * m:128 * (m + 1), :])
            nc.sync.dma_start(bv[:], bv_d[:].bitcast(MMDT['v']))
            for k in range(2):
                nc.sync.dma_start(wp[k][:], wp_d[128 * k:128 * (k + 1), :].bitcast(MMDT['proj']))
            nc.sync.dma_start(tri[:], mask_d[:].bitcast(MMDT['av']))
            nc.sync.dma_start(ones[:], ones_d[:].bitcast(MMDT['v']))

            # ── stage B: qk^T [512, T] = wqk.T @ xt. Two mf tiles x 4 t-chunks
            # fill all 8 PSUM banks per k step, so PE consumes each arriving
            # xt[k] slower than the DMA delivers it (start latency ~= xt[0]). ──
            with tc.tile_pool(name="psB", bufs=1, space="PSUM") as psB:
                for mfp in range(2):
                    pss = {}
                    for mf in (2 * mfp, 2 * mfp + 1):
                        for nt in range(NT):
                            pss[(mf, nt)] = psB.tile(
                                [128, 512], F32, tag=f"psB{mf % 2}_{nt}",
                                name=f"psB{mf}_{nt}")
                    for k in range(KC):
                        for mf in (2 * mfp, 2 * mfp + 1):
                            for nt in range(NT):
                                _mm(nc, pss[(mf, nt)][:],
                                    wqk[k][:, 128 * mf:128 * (mf + 1)],
                                    xt[k][:, 512 * nt:512 * (nt + 1)], "qk",
                                    start=(k == 0), stop=(k == KC - 1))
                    for mf in (2 * mfp, 2 * mfp + 1):
                        for nt in range(NT):
                            nc.vector.tensor_scalar_add(
                                qk[mf][:, 512 * nt:512 * (nt + 1)],
                                pss[(mf, nt)][:], bqk[mf][:])

            # ── stage C: v_aug [T, VW] = xt.T @ wv (+ ones col via K=1) ──
            with tc.tile_pool(name="psC", bufs=3, space="PSUM") as psC:
                for mt in range(T // 128):
                    ps = psC.tile([128, VW], F32, tag="psv", name=f"psv{mt}")
                    for k in range(KC):
                        _mm(nc, ps[:], xt[k][:, 128 * mt:128 * (mt + 1)],
                            wv[k][:], "v", start=(k == 0), stop=False)
                    _mm(nc, ps[:], ones[:, :], bv[:], "k1", start=False,
                        stop=True)
                    nc.vector.tensor_copy(v_sb[mt][:], ps[:])

            # ── stage D: attention. All matmuls keep base partition 0 —
            # alternating base partitions between attT (64-row) and av
            # (128-row) matmuls measured ~1.5x slower on HW. Odd heads'
            # q/k rows live at partitions 64-127, so DMA-shift them down
            # to a base-0 scratch tile first. ──
            LAG = 5   # av lags attT by LAG units to hide the exp latency
            with (
                tc.tile_pool(name="psA", bufs=4, space="PSUM") as psA,
                tc.tile_pool(name="psAV", bufs=1, space="PSUM") as psAV,
                tc.tile_pool(name="expp", bufs=7) as expp,
                tc.tile_pool(name="recp", bufs=2) as recp,
                tc.tile_pool(name="rawp", bufs=2) as rawp,
                tc.tile_pool(name="bcp", bufs=2) as bcp,
                tc.tile_pool(name="shp", bufs=1) as shp,
            ):
                # q/k operands for attT, zero-padded to K=128: rows 0-63
                # hold the head's 64 feature rows, rows 64-127 stay zero so
                # the PE runs the full array (HAM counts it busy) while the
                # zero rows contribute nothing to the contraction.
                def make_qs(h, j):
                    qtile, off = h // 2, 64 * (h % 2)
                    t = shp.tile([128, 512], MMDT["att"], tag="qs",
                                 name=f"qs{h}_{j}", bufs=4)
                    nc.sync.dma_start(t[64:128, :],
                                      zeros_d[:, 0:512].bitcast(MMDT["att"]))
                    nc.sync.dma_start(t[0:64, :],
                                      qk[qtile][off:off + 64,
                                                512 * j:512 * (j + 1)])
                    return t
                def make_ks(h):
                    # bufs=1 + per-chunk DMAs: chunk c of head h+1 only has a
                    # WAR dependency on head h's last reader of chunk c, which
                    # (i-ascending unit order) finishes early -- so the loads
                    # pipeline into the previous head's tail naturally.
                    ktile, off = 2 + h // 2, 64 * (h % 2)
                    t = shp.tile([128, T], MMDT["att"], tag="ks",
                                 name=f"ks{h}", bufs=1)
                    nc.sync.dma_start(t[64:128, :],
                                      zeros_d[:].bitcast(MMDT["att"]))
                    for c in range(NT):
                        nc.sync.dma_start(
                            t[0:64, 512 * c:512 * (c + 1)],
                            qk[ktile][off:off + 64, 512 * c:512 * (c + 1)])
                    return t
                for h in range(HPG):
                    kt_ap = make_ks(h)
                    qs_tiles = [make_qs(h, j) for j in range(NT)]
                    # flat unit order grouped by k-chunk i: the attT stationary
                    # operand (k-tile) and the av stationary operand (v-tile)
                    # are each reused across the j's within a group.
                    units = [(i, j) for i in range(16) for j in range(i // 4, NT)]
                    avp = [psAV.tile([128, 512], F32, tag=f"avj{j}",
                                     name=f"avps{h}_{j}") for j in range(NT)]
                    ets = {}
                    def normalize(j):
                        u = h * NT + j
                        raw = rawp.tile([65, 512], F32, tag="raw",
                                        name=f"raw{h}_{j}")
                        nc.vector.tensor_copy(raw[:], avp[j][0:65, :])
                        nc.sync.dma_start(den_d[u:u + 1, :], raw[64:65, :])
                        den2 = recp.tile([128, 4], F32, tag="den2",
                                         name=f"den2_{h}_{j}")
                        nc.sync.dma_start(
                            den2[:], bass.AP(den_d.tensor, u * 512,
                                             [[4, 128], [1, 4]]))
                        rec2 = recp.tile([128, 4], F32, tag="rec2",
                                         name=f"rec2_{h}_{j}")
                        nc.vector.reciprocal(rec2[:], den2[:])
                        nc.sync.dma_start(
                            bass.AP(rec_d.tensor, u * 512, [[4, 128], [1, 4]]),
                            rec2[:])
                        bc_sb = bcp.tile([64, 512], F32, tag="bc",
                                         name=f"bc{h}_{j}")
                        nc.sync.dma_start(
                            bc_sb[:], bass.AP(rec_d.tensor, u * 512,
                                              [[0, 64], [1, 512]]))
                        off = 64 * (h % 2)
                        nc.vector.tensor_mul(
                            outT[h // 2][off:off + 64, 512 * j:512 * (j + 1)],
                            raw[0:64, :], bc_sb[:])
                    def av_unit(u):
                        i, j = u
                        cc = 128 * (i % 4) if i == 4 * j + (i % 4) and i // 4 == j else 0
                        cc = 128 * (i % 4) if i // 4 == j else 0
                        _mm(nc, avp[j][:, cc:512],
                            v_sb[i][:, 65 * h:65 * h + 128],
                            ets.pop((i, j))[:, cc:512], "av",
                            start=(i == 0), stop=(i == 4 * j + 3))
                    for ui, (i, j) in enumerate(units):
                        diag = (i // 4 == j)
                        c0 = 128 * (i % 4) if diag else 0
                        aps = psA.tile([128, 512], F32, tag="aps",
                                       name=f"aps{h}_{j}_{i}")
                        _mm(nc, aps[:, c0:512],
                            kt_ap[:, 128 * i:128 * (i + 1)],
                            qs_tiles[j][:, c0:512],
                            "att", start=True, stop=True)
                        et = expp.tile([128, 512], MMDT["av"], tag="et",
                                       name=f"et{h}_{j}_{i}")
                        nc.scalar.activation(et[:, c0:512], aps[:, c0:512],
                                             mybir.ActivationFunctionType.Exp)
                        if diag:  # triangular block at cols [c0, c0+128)
                            nc.vector.tensor_mul(et[:, c0:c0 + 128],
                                                 et[:, c0:c0 + 128], tri[:])
                        ets[(i, j)] = et
                        if ui >= LAG:
                            iu, ju = units[ui - LAG]
                            av_unit((iu, ju))
                            if iu == 4 * ju + 3:
                                normalize(ju)
                    for (iu, ju) in units[-LAG:]:
                        av_unit((iu, ju))
                        if iu == 4 * ju + 3:
                            normalize(ju)
            # ── stage E: y^T partial [C, T] = wp.T @ outT ──
            with (
                tc.tile_pool(name="psP", bufs=1, space="PSUM") as psP,
                tc.tile_pool(name="outp", bufs=3) as outp,
            ):
                for mo in range(8):
                    pss = [psP.tile([128, 512], F32, tag=f"psP{nt}",
                                    name=f"psP{mo}_{nt}", bufs=2)
                           for nt in range(NT)]
                    for k in range(2):
                        for nt in range(NT):
                            _mm(nc, pss[nt][:], wp[k][:, 128 * mo:128 * (mo + 1)],
                                outT[k][:, 512 * nt:512 * (nt + 1)], "proj",
                                start=(k == 0), stop=(k == 1))
                    for nt in range(NT):
                        ot = outp.tile([128, 512], F32, tag="ot",
                                       name=f"ot{mo}_{nt}", bufs=4)
                        nc.vector.tensor_copy(ot[:], pss[nt][:])
                        nc.sync.dma_start(
                            yt_d[128 * mo:128 * (mo + 1),
                                 512 * nt:512 * (nt + 1)], ot[:])

            xtp.__exit__(None, None, None)

    nc.compile()
    return nc


def _shard_inputs(x, w_qkv, b_qkv, w_proj):
    scale = 1.0 / np.sqrt(D)   # 0.125, exact power of two
    in_maps = []
    r = np.arange(128)[:, None]
    c = np.arange(128)[None, :]
    mask = np.where(c >= r, 1.0, 0.0).astype(np.float32)
    for core in range(N_CORES):
        b, g = divmod(core, HPG)
        qs = slice(CG * g, CG * (g + 1))
        ks = slice(C + CG * g, C + CG * (g + 1))
        vs = slice(2 * C + CG * g, 2 * C + CG * (g + 1))
        wqk = np.concatenate([w_qkv[qs] * scale, w_qkv[ks]], axis=0).T
        bqk = np.concatenate([b_qkv[qs] * scale, b_qkv[ks]])[:, None]
        wv_base = w_qkv[vs].T          # [C, 256]
        wv = np.zeros((C, VW), np.float32)
        bv = np.zeros((1, VW), np.float32)
        for h in range(HPG):
            wv[:, 65 * h:65 * h + 64] = wv_base[:, 64 * h:64 * h + 64]
            bv[0, 65 * h:65 * h + 64] = b_qkv[vs][64 * h:64 * h + 64]
            bv[0, 65 * h + 64] = 1.0
        in_maps.append({
            "xt": np.ascontiguousarray(x[b].T, np.float32),
            "wqk": np.ascontiguousarray(wqk, np.float32),
            "bqk": np.ascontiguousarray(bqk, np.float32),
            "wv": wv,
            "bv": bv,
            "wp": np.ascontiguousarray(w_proj[:, CG * g:CG * (g + 1)].T,
                                       np.float32),
            "mask": mask,
            "ones": np.ones((1, 128), np.float32),
            "zeros": np.zeros((64, T), np.float32),
        })
    return in_maps


def kernel(x, w_qkv, b_qkv, w_proj, b_proj):
    x = np.asarray(x, np.float32)
    w_qkv = np.asarray(w_qkv, np.float32)
    b_qkv = np.asarray(b_qkv, np.float32)
    w_proj = np.asarray(w_proj, np.float32)
    b_proj = np.asarray(b_proj, np.float32)

    nc = build_nc()
    in_maps = _shard_inputs(x, w_qkv, b_qkv, w_proj)
    if _trace_flag[0]:
        _ensure_ntff_hook()
    res = run_bass_kernel_spmd(nc, in_maps, core_ids=list(range(N_CORES)),
                               trace=_trace_flag[0])
    _last_results[0] = res

    y = np.empty((B, T, C), np.float32)
    for b in range(B):
        acc = np.zeros((C, T), np.float32)
        for g in range(HPG):
            acc += res.results[HPG * b + g]["yt"]
        y[b] = acc.T + b_proj[None, :]
    return y



# revision 2
# speedup vs baseline: 1.1615x; 1.1615x over previous
"""Causal self-attention (B=2, T=2048, C=1024, H=16, D=64) on 8 trn2 cores. v2.

Sharding: core c -> batch b = c // 4, head-group g = c % 4 (4 heads each).

Mixed precision design (validated vs reference in numpy, ~1.1e-2 rel):
  q/k projection : fp8e4 DoubleRow matmuls (x, Wq, Wk in e4m3; W scaled x64)
  v projection   : bf16 matmuls (fp8 v fails the early-token error budget)
  scores (attT)  : fp8e4 operands for q-chunks >=1; bf16 for q-chunk 0
  softmax exp    : q-chunk 0 -> ACT real exp -> bf16 P
                   q-chunks >=1 -> DVE/ACT int8 "bit-trick": byte =
                   round(logit*8/ln2 + 56) interpreted as e4m3 == e^z*(1+-4%)
                   (int8 cast saturates; masked cols -> -128 = -0.0 in e4m3)
  P @ V (AV)     : q-chunk 0: bf16; q-chunks >=1: fp8 DoubleRow over k-chunk
                   pairs (P pair tiles [128, 2*512], v pairs strided in one tile)
  denominator    : ones-column in v_aug; DMA psum row 64 -> DRAM, strided
                   reload, DVE reciprocal, partition-broadcast reload
  out projection : bf16, psum evacuated by ACT/DVE alternately -> bf16 DMA out
"""

import os
import sys
import types

for _p in ("/opt/trn_rl_repo", "/root/.axon_site", "/root/.axon_site/_ro/trn_rl_repo"):
    if os.path.isdir(_p) and _p not in sys.path:
        sys.path.append(_p)

import numpy as np
import ml_dtypes

import concourse.bacc as bacc
import concourse.bass as bass
import concourse.mybir as mybir
import concourse.tile as tile
from concourse.bass_utils import run_bass_kernel_spmd

B, T, C = 2, 2048, 1024
H, D = 16, 64
N_CORES = 8
HPG = 4                 # heads per group (per core)
CG = HPG * D            # 256 channels per head-group
NT = T // 512           # 4 q-chunks of 512
KC = C // 128           # 8 contraction tiles over C
VW = HPG * 65 + 64      # v tile width per token chunk

F32 = mybir.dt.float32
BF16 = mybir.dt.bfloat16
FP8 = mybir.dt.float8e4
I8 = mybir.dt.int8

LN2 = float(np.log(2.0))
WSCALE = 64.0                      # fp8 storage scale for Wq/Wk
A_BYTE = 8.0 / (LN2 * (WSCALE * WSCALE * 8.0))   # logit_true = raw/ (64*64*8)
B_BYTE = 56.0                      # e4m3 byte of 1.0
MASKV = -1e4                       # additive mask (saturates int8 to -128 = -0.0)

DoubleRow = mybir.MatmulPerfMode.DoubleRow
Exp = mybir.ActivationFunctionType.Exp
Identity = mybir.ActivationFunctionType.Identity
Copy = mybir.ActivationFunctionType.Copy
MULT = mybir.AluOpType.mult
ADD = mybir.AluOpType.add

_trace_flag = [False]
_last_results = [None]


def _ensure_ntff_hook():
    if "antenv.axon_hooks" in sys.modules:
        return
    try:
        from trn_agent_boot.trn_boot import _ntff_profile_via_ctypes
    except Exception:
        return
    mod = types.ModuleType("antenv.axon_hooks")
    hook = [None]
    mod.set_axon_ntff_profile_hook = lambda h: hook.__setitem__(0, h)
    mod.get_axon_ntff_profile_hook = lambda: hook[0]
    sys.modules["antenv.axon_hooks"] = mod
    so = "/opt/axon/libaxon_pjrt.so"
    if os.path.exists(so):
        mod.set_axon_ntff_profile_hook(_ntff_profile_via_ctypes(so))


def build_nc():
    nc = bacc.Bacc("TRN2", target_bir_lowering=False, debug=False,
                   num_devices=N_CORES)

    xt8_d = nc.dram_tensor("xt8", [C, T], FP8, kind="ExternalInput").ap()
    xtb_d = nc.dram_tensor("xtb", [C, T], BF16, kind="ExternalInput").ap()
    wqk8_d = nc.dram_tensor("wqk8", [128, 32 * 128], FP8, kind="ExternalInput").ap()
    bqk64_d = nc.dram_tensor("bqk64", [2 * CG, 1], F32, kind="ExternalInput").ap()
    bqkt_d = nc.dram_tensor("bqkt", [2 * CG, 1], F32, kind="ExternalInput").ap()
    wvb_d = nc.dram_tensor("wvb", [C, VW], BF16, kind="ExternalInput").ap()
    bvb_d = nc.dram_tensor("bvb", [1, VW], BF16, kind="ExternalInput").ap()
    wpb_d = nc.dram_tensor("wpb", [CG, C], BF16, kind="ExternalInput").ap()
    maskA_d = nc.dram_tensor("maskA", [128, 1024], F32, kind="ExternalInput").ap()
    maskB_d = nc.dram_tensor("maskB", [128, 512], F32, kind="ExternalInput").ap()
    trib_d = nc.dram_tensor("trib", [128, 128], BF16, kind="ExternalInput").ap()
    yt_d = nc.dram_tensor("yt", [C, T], BF16, kind="ExternalOutput").ap()
    den_d = nc.dram_tensor("den_scratch", [HPG * NT, 512], F32).ap()
    rec_d = nc.dram_tensor("rec_scratch", [HPG * NT, 512], F32).ap()

    with tile.TileContext(nc) as tc:
        with tc.tile_pool(name="const", bufs=1) as cp:
            # ---- persistent SBUF residents ----
            xt8 = cp.tile([128, KC * T], FP8, tag="xt8")          # 16 KB/p
            xtb = cp.tile([128, KC * T], BF16, tag="xtb")         # 32 KB/p
            wqk8 = cp.tile([128, KC * 2 * CG], FP8, tag="wqk8")   # 4 KB/p
            bqk64 = [cp.tile([128, 1], F32, tag=f"bqk64_{m}", name=f"bqk64_{m}") for m in range(4)]
            bqkt = [cp.tile([128, 1], F32, tag=f"bqkt_{m}", name=f"bqkt_{m}") for m in range(4)]
            wvb = cp.tile([128, KC * VW], BF16, tag="wvb")        # 5.2 KB/p
            bvb = cp.tile([1, VW], BF16, tag="bvb")
            onesb = cp.tile([1, 128], BF16, tag="onesb")
            wpb = [cp.tile([128, C], BF16, tag=f"wpb{k}", name=f"wpb{k}") for k in range(2)]
            maskA = cp.tile([128, 1024], F32, tag="maskA")
            maskB = cp.tile([128, 512], F32, tag="maskB")
            trib = cp.tile([128, 128], BF16, tag="trib")
            # qk8[m]: m=0: q heads01, 1: q heads23, 2: k heads01, 3: k heads23
            qk8 = [cp.tile([128, T], FP8, tag=f"qk8_{m}", name=f"qk8_{m}") for m in range(4)]
            # bf16 true-scale q/k, q-chunk 0 only (cols 0:512 of q; k all cols)
            qkb = [cp.tile([128, 512], BF16, tag=f"qkb_{m}", name=f"qkb_{m}") for m in range(2)]
            kkb = [cp.tile([128, 512], BF16, tag=f"kkb_{m}", name=f"kkb_{m}") for m in range(2)]
            v8 = cp.tile([128, (T // 128) * VW], FP8, tag="v8")   # 5.1 KB/p
            v8p = cp.tile([128, 8 * 4 * 256], FP8, tag="v8p")    # 8 KB/p
            vb = cp.tile([128, 4 * VW], BF16, tag="vb")           # chunks 0-3
            outT = [cp.tile([128, T], BF16, tag=f"outT{k}", name=f"outT{k}") for k in range(2)]

            # staging tiles (persistent; zero rows written once)
            ks8 = [cp.tile([128, T], FP8, tag=f"ks8_{i}", name=f"ks8_{i}") for i in range(2)]
            qs8 = [cp.tile([128, 512], FP8, tag=f"qs8_{i}", name=f"qs8_{i}") for i in range(6)]
            ksb = [cp.tile([128, 512], BF16, tag=f"ksb_{i}", name=f"ksb_{i}") for i in range(2)]
            qsb = [cp.tile([128, 512], BF16, tag=f"qsb_{i}", name=f"qsb_{i}") for i in range(2)]

            xt8_r = xt8[:].rearrange("p (k n) -> p k n", k=KC)

            # ---- input DMA: interleave wqk8/xt8 pairs (stage B critical path),
            # then everything else ----
            for kp in range(KC // 2):
                nc.sync.dma_start(wqk8[:, 1024 * kp:1024 * (kp + 1)],
                                  wqk8_d[:, 1024 * kp:1024 * (kp + 1)])
                nc.sync.dma_start(xt8[:, T * 2 * kp:T * (2 * kp + 1)],
                                  xt8_d[128 * 2 * kp:128 * (2 * kp + 1), :])
                nc.sync.dma_start(xt8[:, T * (2 * kp + 1):T * (2 * kp + 2)],
                                  xt8_d[128 * (2 * kp + 1):128 * (2 * kp + 2), :])
            for m in range(4):
                nc.sync.dma_start(bqk64[m][:], bqk64_d[128 * m:128 * (m + 1), :])
                nc.sync.dma_start(bqkt[m][:], bqkt_d[128 * m:128 * (m + 1), :])
            for kk in range(KC):
                nc.sync.dma_start(xtb[:, T * kk:T * (kk + 1)],
                                  xtb_d[128 * kk:128 * (kk + 1), :])
            for kk in range(KC):
                nc.sync.dma_start(wvb[:, VW * kk:VW * (kk + 1)],
                                  wvb_d[128 * kk:128 * (kk + 1), :])
            nc.sync.dma_start(bvb[:], bvb_d[:])
            for k in range(2):
                nc.sync.dma_start(wpb[k][:], wpb_d[128 * k:128 * (k + 1), :])
            nc.sync.dma_start(maskA[:], maskA_d[:])
            nc.sync.dma_start(maskB[:], maskB_d[:])
            nc.sync.dma_start(trib[:], trib_d[:])
            nc.vector.memset(onesb[:], 1.0)
            # zero the pad rows of the staging tiles once
            for t in ks8:
                nc.gpsimd.memset(t[64:128, :], 0)
            for t in qs8:
                nc.gpsimd.memset(t[64:128, :], 0)
            for t in ksb:
                nc.gpsimd.memset(t[64:128, :], 0)
            for t in qsb:
                nc.gpsimd.memset(t[64:128, :], 0)

            # ---- stage B: q/k projection, fp8 DoubleRow ----
            # out channels: [q(0:256), k(256:512)]; mf chunks of 128.
            # group A = (mf0, mf2) -> heads 0,1 done first.
            with tc.tile_pool(name="psB", bufs=1, space="PSUM") as psB:
                for grp in ([0, 2], [1, 3]):
                    pss = {}
                    for mf in grp:
                        for nt in range(NT):
                            pss[(mf, nt)] = psB.tile(
                                [128, 512], F32, tag=f"psB{grp.index(mf)}_{nt}",
                                name=f"psB{mf}_{nt}")
                    for kp in range(KC // 2):
                        for mf in grp:
                            for nt in range(NT):
                                wb = (kp * 8 + mf * 2) * 128
                                nc.tensor.matmul(
                                    pss[(mf, nt)][:],
                                    wqk8[:, wb:wb + 256].rearrange(
                                        "p (two m) -> p two m", two=2),
                                    xt8_r[:, 2 * kp:2 * kp + 2,
                                          512 * nt:512 * (nt + 1)],
                                    start=(kp == 0), stop=(kp == KC // 2 - 1),
                                    perf_mode=DoubleRow)
                    for mf in grp:
                        for nt in range(NT):
                            ps = pss[(mf, nt)]
                            # fp8 q'/k' at 64x scale (+64x bias)
                            nc.vector.tensor_scalar(
                                qk8[mf][:, 512 * nt:512 * (nt + 1)], ps[:],
                                1.0, bqk64[mf][:], op0=MULT, op1=ADD)
                            # bf16 true-scale for the j0 path
                            if mf in (0, 1) and nt == 0:
                                nc.scalar.activation(
                                    qkb[mf][:], ps[:], Identity,
                                    bias=bqkt[mf][:], scale=1.0 / WSCALE)
                            if mf in (2, 3) and nt == 0:
                                nc.scalar.activation(
                                    kkb[mf - 2][:], ps[:], Identity,
                                    bias=bqkt[mf][:], scale=1.0 / WSCALE)

            # ---- stage C: v projection, bf16 (+ ones col via K=1 bias matmul) ----
            with tc.tile_pool(name="psC", bufs=3, space="PSUM") as psC:
                for mt in range(T // 128):
                    ps = psC.tile([128, VW], F32, tag="psv", name=f"psv{mt}")
                    for kk in range(KC):
                        nc.tensor.matmul(
                            ps[:],
                            xtb[:, T * kk + 128 * mt:T * kk + 128 * (mt + 1)],
                            wvb[:, VW * kk:VW * (kk + 1)],
                            start=(kk == 0), stop=False)
                    nc.tensor.matmul(ps[:], onesb[:, :], bvb[:],
                                     start=False, stop=True)
                    nc.vector.tensor_copy(v8[:, VW * mt:VW * (mt + 1)], ps[:])
                    if mt < 4:
                        nc.scalar.activation(vb[:, VW * mt:VW * (mt + 1)],
                                             ps[:], Copy)
                    if mt % 2 == 1:
                        pp = mt // 2
                        for hh in range(HPG):
                            for s in range(2):
                                dst = ((pp * 4 + hh) * 2 + s) * 128
                                nc.sync.dma_start(
                                    v8p[:, dst:dst + 128],
                                    v8[:, VW * (2 * pp + s) + 65 * hh:
                                       VW * (2 * pp + s) + 65 * hh + 128])

            # ---- stage D: attention ----
            # pair schedule per head: j0 (bf16) pairs then j>=1 (fp8 DR) pairs
            with (
                tc.tile_pool(name="psA", bufs=2, space="PSUM") as psA,
                tc.tile_pool(name="psAV", bufs=1, space="PSUM") as psAV,
                tc.tile_pool(name="etp", bufs=4) as etp,
                tc.tile_pool(name="et0p", bufs=2) as et0p,
                tc.tile_pool(name="recp", bufs=2) as recp,
                tc.tile_pool(name="bcp", bufs=2) as bcp,
            ):
                act_load = [0.0]   # running col-count per engine for balance
                dve_load = [0.0]

                for h in range(HPG):
                    qtile, off = h // 2, 64 * (h % 2)
                    ktile = 2 + h // 2
                    # --- staging DMAs ---
                    ks = ks8[h % 2]
                    for cch in range(NT):
                        nc.sync.dma_start(
                            ks[0:64, 512 * cch:512 * (cch + 1)],
                            qk8[ktile][off:off + 64, 512 * cch:512 * (cch + 1)])
                    kb = ksb[h % 2]
                    nc.sync.dma_start(kb[0:64, :], kkb[h // 2][off:off + 64, 0:512])
                    qb = qsb[h % 2]
                    nc.sync.dma_start(qb[0:64, :], qkb[h // 2][off:off + 64, :])
                    qss = {}
                    for j in range(1, NT):
                        t = qs8[(h % 2) * 3 + (j - 1)]
                        nc.sync.dma_start(
                            t[0:64, :],
                            qk8[qtile][off:off + 64, 512 * j:512 * (j + 1)])
                        qss[j] = t

                    avp = [psAV.tile([128, 512], F32, tag=f"avj{j}",
                                     name=f"avps{h}_{j}") for j in range(NT)]

                    # pair list: (j, p, c0pair, kind)
                    pairs = []
                    pairs.append((0, 0, 0, "j0"))
                    pairs.append((0, 1, 256, "j0"))
                    for j in range(1, NT):
                        for p in range(2 * j + 2):
                            i0 = 2 * p
                            if i0 == 4 * j:
                                pairs.append((j, p, 0, "dA"))
                            elif i0 == 4 * j + 2:
                                pairs.append((j, p, 256, "dB"))
                            else:
                                pairs.append((j, p, 0, "nd"))

                    pending_av = []

                    def emit_av(rec):
                        j, p, c0p, kind, et = rec
                        if kind == "j0":
                            for s in range(2):
                                i = 2 * p + s
                                cc = 128 * i
                                nc.tensor.matmul(
                                    avp[0][:, cc:512],
                                    vb[:, VW * i + 65 * h:VW * i + 65 * h + 128],
                                    et[:, 512 * s + cc:512 * s + 512],
                                    start=(i == 0), stop=(i == 3))
                            if p == 1:
                                normalize(h, 0)
                        else:
                            if c0p == 0:
                                et_r = et[:].bitcast(FP8).rearrange(
                                    "p (two n) -> p two n", two=2)
                            else:
                                et_r = et[:].bitcast(FP8)[
                                    :, 256:768].rearrange(
                                    "p (two n) -> p two n", two=2)
                            vbase = ((p * 4 + h) * 2) * 128
                            nc.tensor.matmul(
                                avp[j][:, c0p:512],
                                v8p[:, vbase:vbase + 256].rearrange(
                                    "p (two m) -> p two m", two=2),
                                et_r,
                                start=(p == 0), stop=(p == 2 * j + 1),
                                perf_mode=DoubleRow)
                            if p == 2 * j + 1:
                                normalize(h, j)

                    def normalize(h_, j_):
                        u = h_ * NT + j_
                        rowb = recp.tile([1, 512], F32, tag="rowb",
                                         name=f"rowb{h_}_{j_}")
                        if (h_ + j_) % 2 == 0:
                            nc.scalar.activation(rowb[:], avp[j_][64:65, :],
                                                 Copy)
                        else:
                            nc.vector.tensor_copy(rowb[:], avp[j_][64:65, :])
                        nc.sync.dma_start(den_d[u:u + 1, :], rowb[:])
                        den2 = recp.tile([128, 4], F32, tag="den2",
                                         name=f"den2_{h_}_{j_}")
                        nc.sync.dma_start(
                            den2[:], bass.AP(den_d.tensor, u * 512,
                                             [[4, 128], [1, 4]]))
                        rec2 = recp.tile([128, 4], F32, tag="rec2",
                                         name=f"rec2_{h_}_{j_}")
                        nc.vector.reciprocal(rec2[:], den2[:])
                        nc.sync.dma_start(
                            bass.AP(rec_d.tensor, u * 512, [[4, 128], [1, 4]]),
                            rec2[:])
                        bc = bcp.tile([64, 512], F32, tag="bc",
                                      name=f"bc{h_}_{j_}")
                        nc.sync.dma_start(
                            bc[:], bass.AP(rec_d.tensor, u * 512,
                                           [[0, 64], [1, 512]]))
                        off_ = 64 * (h_ % 2)
                        nc.vector.scalar_tensor_tensor(
                            outT[h_ // 2][off_:off_ + 64,
                                          512 * j_:512 * (j_ + 1)],
                            avp[j_][0:64, :], 1.0, bc[:],
                            op0=MULT, op1=MULT)

                    LAG = 2
                    for ui, (j, p, c0p, kind) in enumerate(pairs):
                        # attT pair -> psum pair tile [128, 1024]
                        aps = psA.tile([128, 1024], F32, tag="aps",
                                       name=f"aps{h}_{j}_{p}")
                        if kind == "j0":
                            et = et0p.tile([128, 1024], BF16, tag="et0",
                                           name=f"et0_{h}_{p}")
                            for s in range(2):
                                i = 2 * p + s
                                cc = 128 * i
                                nc.tensor.matmul(
                                    aps[:, 512 * s + cc:512 * s + 512],
                                    kb[:, 128 * i:128 * (i + 1)],
                                    qb[:, cc:512], start=True, stop=True)
                                nc.scalar.activation(
                                    et[:, 512 * s + cc:512 * s + 512],
                                    aps[:, 512 * s + cc:512 * s + 512],
                                    Exp, scale=0.125)
                                act_load[0] += 512 - cc
                                # mask diag block via Pool multiply
                                nc.gpsimd.tensor_mul(
                                    et[:, 512 * s + cc:512 * s + cc + 128],
                                    et[:, 512 * s + cc:512 * s + cc + 128],
                                    trib[:])
                        else:
                            et = etp.tile([128, 1024], I8, tag="et",
                                          name=f"et_{h}_{j}_{p}")
                            for s in range(2):
                                i = 2 * p + s
                                c0u = c0p if kind == "nd" else (
                                    128 * ((2 * p + s) % 4))
                                nc.tensor.matmul(
                                    aps[:, 512 * s + c0u:512 * s + 512],
                                    ks[:, 128 * i:128 * (i + 1)],
                                    qss[j][:, c0u:512], start=True, stop=True)
                            if kind == "nd":
                                # whole pair, no mask: cheaper engine does it
                                if act_load[0] <= dve_load[0]:
                                    nc.scalar.activation(
                                        et[:].bitcast(FP8), aps[:], Exp,
                                        scale=A_BYTE * LN2 / 8.0)
                                    act_load[0] += 1024
                                else:
                                    nc.vector.tensor_scalar(
                                        et[:], aps[:], A_BYTE, B_BYTE,
                                        op0=MULT, op1=ADD)
                                    dve_load[0] += 1024
                            elif kind == "dA":
                                nc.vector.scalar_tensor_tensor(
                                    et[:], aps[:], A_BYTE, maskA[:],
                                    op0=MULT, op1=ADD)
                                dve_load[0] += 1024
                            else:  # dB: psum ranges 256-511 & 768-1023,
                                # compacted out to et [256:768)
                                aps_r = aps[:].rearrange(
                                    "p (k n) -> p k n", k=4)[:, 1:4:2, :]
                                et_r2 = et[:, 256:768].rearrange(
                                    "p (k n) -> p k n", k=2)
                                nc.vector.scalar_tensor_tensor(
                                    et_r2, aps_r, A_BYTE,
                                    maskB[:].rearrange(
                                        "p (k n) -> p k n", k=2),
                                    op0=MULT, op1=ADD)
                                dve_load[0] += 512
                        pending_av.append((j, p, c0p, kind, et))
                        if len(pending_av) > LAG:
                            emit_av(pending_av.pop(0))
                    while pending_av:
                        emit_av(pending_av.pop(0))

            # ---- stage E: y^T partial = wp.T @ outT, bf16 ----
            with (
                tc.tile_pool(name="psP", bufs=2, space="PSUM") as psP,
                tc.tile_pool(name="outp", bufs=4) as outp,
            ):
                for mo in range(8):
                    pss = [psP.tile([128, 512], F32, tag=f"psP{nt}",
                                    name=f"psP{mo}_{nt}") for nt in range(NT)]
                    for k in range(2):
                        for nt in range(NT):
                            nc.tensor.matmul(
                                pss[nt][:], wpb[k][:, 128 * mo:128 * (mo + 1)],
                                outT[k][:, 512 * nt:512 * (nt + 1)],
                                start=(k == 0), stop=(k == 1))
                    for nt in range(NT):
                        ot = outp.tile([128, 512], BF16, tag="ot",
                                       name=f"ot{mo}_{nt}")
                        if nt % 2 == 0:
                            nc.vector.tensor_copy(ot[:], pss[nt][:])
                        else:
                            nc.scalar.activation(ot[:], pss[nt][:], Copy)
                        nc.sync.dma_start(
                            yt_d[128 * mo:128 * (mo + 1),
                                 512 * nt:512 * (nt + 1)], ot[:])

    nc.compile()
    return nc


def _shard_inputs(x, w_qkv, b_qkv, w_proj):
    e4 = ml_dtypes.float8_e4m3fn
    bf = ml_dtypes.bfloat16
    in_maps = []
    r = np.arange(128)[:, None]
    c = np.arange(128)[None, :]
    tri01 = (c >= r).astype(np.float32)            # keep iff q >= k
    # byte-domain additive masks: keep -> 56.0, masked -> -1e4
    def seg_tri(q0):
        # [128, 128] tri: rows k-local, cols q-local, keep iff (q0+c) >= r
        return np.where(c >= r, B_BYTE, MASKV).astype(np.float32)
    keep = np.full((128, 128), B_BYTE, np.float32)
    full = np.full((128, 128), MASKV, np.float32)
    maskA = np.concatenate(
        [seg_tri(0), keep, keep, keep,            # slot0: tri@0, open
         full, seg_tri(0), keep, keep], axis=1)   # slot1: full@0, tri@128, open
    maskB = np.concatenate(
        [seg_tri(0), keep,                        # slot0 (cols 256-511): tri, open
         full, seg_tri(0)], axis=1)               # slot1: full, tri
    trib = tri01.astype(bf)

    for core in range(N_CORES):
        b, g = divmod(core, HPG)
        qs = slice(CG * g, CG * (g + 1))
        ks = slice(C + CG * g, C + CG * (g + 1))
        vs = slice(2 * C + CG * g, 2 * C + CG * (g + 1))
        # wqk8: [C, 512] = 64 * [Wq; Wk]^T in e4m3 (no attention scale folded;
        # it lives in A_BYTE / the j0 exp scale)
        wqk = np.concatenate([w_qkv[qs] * WSCALE, w_qkv[ks] * WSCALE],
                             axis=0).T
        # paired dual-fp8 layout: [p, kp, mf, s, m] -> [128, 4096]
        wqk = np.ascontiguousarray(
            wqk.reshape(4, 2, 128, 4, 128).transpose(2, 0, 3, 1, 4)
            .reshape(128, 4096))
        bqk64 = (np.concatenate([b_qkv[qs], b_qkv[ks]]) * WSCALE)[:, None]
        bqkt = np.concatenate([b_qkv[qs], b_qkv[ks]])[:, None]
        wv_base = w_qkv[vs].T
        wv = np.zeros((C, VW), np.float32)
        bv = np.zeros((1, VW), np.float32)
        for h in range(HPG):
            wv[:, 65 * h:65 * h + 64] = wv_base[:, 64 * h:64 * h + 64]
            bv[0, 65 * h:65 * h + 64] = b_qkv[vs][64 * h:64 * h + 64]
            bv[0, 65 * h + 64] = 1.0
        xt = np.ascontiguousarray(x[b].T, np.float32)
        in_maps.append({
            "xt8": xt.astype(e4),
            "xtb": xt.astype(bf),
            "wqk8": np.ascontiguousarray(wqk).astype(e4),
            "bqk64": np.ascontiguousarray(bqk64, np.float32),
            "bqkt": np.ascontiguousarray(bqkt, np.float32),
            "wvb": wv.astype(bf),
            "bvb": bv.astype(bf),
            "wpb": np.ascontiguousarray(
                w_proj[:, CG * g:CG * (g + 1)].T).astype(bf),
            "maskA": maskA,
            "maskB": maskB,
            "trib": trib,
        })
    return in_maps


def kernel(x, w_qkv, b_qkv, w_proj, b_proj):
    x = np.asarray(x, np.float32)
    w_qkv = np.asarray(w_qkv, np.float32)
    b_qkv = np.asarray(b_qkv, np.float32)
    w_proj = np.asarray(w_proj, np.float32)
    b_proj = np.asarray(b_proj, np.float32)

    nc = build_nc()
    in_maps = _shard_inputs(x, w_qkv, b_qkv, w_proj)
    if _trace_flag[0]:
        _ensure_ntff_hook()
    res = run_bass_kernel_spmd(nc, in_maps, core_ids=list(range(N_CORES)),
                               trace=_trace_flag[0])
    _last_results[0] = res

    y = np.empty((B, T, C), np.float32)
    for b in range(B):
        acc = np.zeros((C, T), np.float32)
        for g in range(HPG):
            acc += np.asarray(res.results[HPG * b + g]["yt"], np.float32)
        y[b] = acc.T + b_proj[None, :]
    return y


# revision 3
# speedup vs baseline: 1.2105x; 1.0422x over previous
"""Causal self-attention (B=2, T=2048, C=1024, H=16, D=64) on 8 trn2 cores. v2.

Sharding: core c -> batch b = c // 4, head-group g = c % 4 (4 heads each).

Mixed precision design (validated vs reference in numpy, ~1.1e-2 rel):
  q/k projection : fp8e4 DoubleRow matmuls (x, Wq, Wk in e4m3; W scaled x64)
  v projection   : bf16 matmuls (fp8 v fails the early-token error budget)
  scores (attT)  : fp8e4 operands for q-chunks >=1; bf16 for q-chunk 0
  softmax exp    : q-chunk 0 -> ACT real exp -> bf16 P
                   q-chunks >=1 -> DVE/ACT int8 "bit-trick": byte =
                   round(logit*8/ln2 + 56) interpreted as e4m3 == e^z*(1+-4%)
                   (int8 cast saturates; masked cols -> -128 = -0.0 in e4m3)
  P @ V (AV)     : q-chunk 0: bf16; q-chunks >=1: fp8 DoubleRow over k-chunk
                   pairs (P pair tiles [128, 2*512], v pairs strided in one tile)
  denominator    : ones-column in v_aug; DMA psum row 64 -> DRAM, strided
                   reload, DVE reciprocal, partition-broadcast reload
  out projection : bf16, psum evacuated by ACT/DVE alternately -> bf16 DMA out
"""

import os
import sys
import types

for _p in ("/opt/trn_rl_repo", "/root/.axon_site", "/root/.axon_site/_ro/trn_rl_repo"):
    if os.path.isdir(_p) and _p not in sys.path:
        sys.path.append(_p)

import numpy as np
import ml_dtypes

import concourse.bacc as bacc
import concourse.bass as bass
import concourse.mybir as mybir
import concourse.tile as tile
from concourse.bass_utils import run_bass_kernel_spmd

B, T, C = 2, 2048, 1024
H, D = 16, 64
N_CORES = 8
HPG = 4                 # heads per group (per core)
CG = HPG * D            # 256 channels per head-group
NT = T // 512           # 4 q-chunks of 512
KC = C // 128           # 8 contraction tiles over C
VW = HPG * 65 + 64      # v tile width per token chunk

F32 = mybir.dt.float32
BF16 = mybir.dt.bfloat16
FP8 = mybir.dt.float8e4
I8 = mybir.dt.int8

LN2 = float(np.log(2.0))
WSCALE = 64.0                      # fp8 storage scale for Wq/Wk
A_BYTE = 8.0 / (LN2 * (WSCALE * WSCALE * 8.0))   # logit_true = raw/ (64*64*8)
B_BYTE = 56.0                      # e4m3 byte of 1.0
MASKV = -1e4                       # additive mask (saturates int8 to -128 = -0.0)

DoubleRow = mybir.MatmulPerfMode.DoubleRow
Exp = mybir.ActivationFunctionType.Exp
Identity = mybir.ActivationFunctionType.Identity
Copy = mybir.ActivationFunctionType.Copy
MULT = mybir.AluOpType.mult
ADD = mybir.AluOpType.add

_trace_flag = [False]
_last_results = [None]


def _ensure_ntff_hook():
    if "antenv.axon_hooks" in sys.modules:
        return
    try:
        from trn_agent_boot.trn_boot import _ntff_profile_via_ctypes
    except Exception:
        return
    mod = types.ModuleType("antenv.axon_hooks")
    hook = [None]
    mod.set_axon_ntff_profile_hook = lambda h: hook.__setitem__(0, h)
    mod.get_axon_ntff_profile_hook = lambda: hook[0]
    sys.modules["antenv.axon_hooks"] = mod
    so = "/opt/axon/libaxon_pjrt.so"
    if os.path.exists(so):
        mod.set_axon_ntff_profile_hook(_ntff_profile_via_ctypes(so))


def build_nc():
    nc = bacc.Bacc("TRN2", target_bir_lowering=False, debug=False,
                   num_devices=N_CORES)

    xt8_d = nc.dram_tensor("xt8", [C, T], FP8, kind="ExternalInput").ap()
    xtb_d = nc.dram_tensor("xtb", [C, T], BF16, kind="ExternalInput").ap()
    wqk8_d = nc.dram_tensor("wqk8", [128, 32 * 128], FP8, kind="ExternalInput").ap()
    bqk64_d = nc.dram_tensor("bqk64", [2 * CG, 1], F32, kind="ExternalInput").ap()
    bqkt_d = nc.dram_tensor("bqkt", [2 * CG, 1], F32, kind="ExternalInput").ap()
    wvb_d = nc.dram_tensor("wvb", [C, VW], BF16, kind="ExternalInput").ap()
    bvb_d = nc.dram_tensor("bvb", [1, VW], BF16, kind="ExternalInput").ap()
    wpb_d = nc.dram_tensor("wpb", [CG, C], BF16, kind="ExternalInput").ap()
    maskA_d = nc.dram_tensor("maskA", [128, 1024], F32, kind="ExternalInput").ap()
    maskB_d = nc.dram_tensor("maskB", [128, 512], F32, kind="ExternalInput").ap()
    trib_d = nc.dram_tensor("trib", [128, 128], BF16, kind="ExternalInput").ap()
    yt_d = nc.dram_tensor("yt", [C, T], BF16, kind="ExternalOutput").ap()
    den_d = nc.dram_tensor("den_scratch", [HPG * NT, 512], F32).ap()
    rec_d = nc.dram_tensor("rec_scratch", [HPG * NT, 512], F32).ap()

    with tile.TileContext(nc) as tc:
        with tc.tile_pool(name="const", bufs=1) as cp:
            # ---- persistent SBUF residents ----
            xt8 = cp.tile([128, KC * T], FP8, tag="xt8")          # 16 KB/p
            xtb = cp.tile([128, KC * T], BF16, tag="xtb")         # 32 KB/p
            wqk8 = cp.tile([128, KC * 2 * CG], FP8, tag="wqk8")   # 4 KB/p
            bqk64 = [cp.tile([128, 1], F32, tag=f"bqk64_{m}", name=f"bqk64_{m}") for m in range(4)]
            bqkt = [cp.tile([128, 1], F32, tag=f"bqkt_{m}", name=f"bqkt_{m}") for m in range(4)]
            wvb = cp.tile([128, KC * VW], BF16, tag="wvb")        # 5.2 KB/p
            bvb = cp.tile([1, VW], BF16, tag="bvb")
            onesb = cp.tile([1, 128], BF16, tag="onesb")
            wpb = [cp.tile([128, C], BF16, tag=f"wpb{k}", name=f"wpb{k}") for k in range(2)]
            maskA = cp.tile([128, 1024], F32, tag="maskA")
            maskB = cp.tile([128, 512], F32, tag="maskB")
            trib = cp.tile([128, 128], BF16, tag="trib")
            # qk8[m]: m=0: q heads01, 1: q heads23, 2: k heads01, 3: k heads23
            qk8 = [cp.tile([128, T], FP8, tag=f"qk8_{m}", name=f"qk8_{m}") for m in range(4)]
            # bf16 true-scale q/k, q-chunk 0 only (cols 0:512 of q; k all cols)
            qkb = [cp.tile([128, 512], BF16, tag=f"qkb_{m}", name=f"qkb_{m}") for m in range(2)]
            kkb = [cp.tile([128, 512], BF16, tag=f"kkb_{m}", name=f"kkb_{m}") for m in range(2)]
            v8 = cp.tile([128, (T // 128) * VW], FP8, tag="v8")   # 5.1 KB/p
            v8p = cp.tile([128, 8 * 4 * 256], FP8, tag="v8p")    # 8 KB/p
            vb = cp.tile([128, 4 * VW], BF16, tag="vb")           # chunks 0-3
            outT = [cp.tile([128, T], BF16, tag=f"outT{k}", name=f"outT{k}") for k in range(2)]

            # staging tiles (persistent; zero rows written once)
            ks8 = [cp.tile([128, T], FP8, tag=f"ks8_{i}", name=f"ks8_{i}") for i in range(2)]
            qs8 = [cp.tile([128, 512], FP8, tag=f"qs8_{i}", name=f"qs8_{i}") for i in range(6)]
            ksb = [cp.tile([128, 512], BF16, tag=f"ksb_{i}", name=f"ksb_{i}") for i in range(2)]
            qsb = [cp.tile([128, 512], BF16, tag=f"qsb_{i}", name=f"qsb_{i}") for i in range(2)]

            xt8_r = xt8[:].rearrange("p (k n) -> p k n", k=KC)

            # ---- input DMA: interleave wqk8/xt8 pairs (stage B critical path),
            # then everything else ----
            nc.sync.dma_start(wqk8[:], wqk8_d[:])
            for kp in range(KC // 2):
                nc.sync.dma_start(
                    xt8[:].rearrange("p (k n) -> p k n", k=KC)[:, 2 * kp:2 * kp + 2, :],
                    xt8_d.rearrange("(k p) n -> p k n", k=KC)[:, 2 * kp:2 * kp + 2, :])
            for m in range(4):
                nc.sync.dma_start(bqk64[m][:], bqk64_d[128 * m:128 * (m + 1), :])
                nc.sync.dma_start(bqkt[m][:], bqkt_d[128 * m:128 * (m + 1), :])
            nc.sync.dma_start(wvb[:].rearrange("p (k n) -> p k n", k=KC),
                              wvb_d.rearrange("(k p) n -> p k n", k=KC))
            nc.sync.dma_start(bvb[:], bvb_d[:])
            for half in range(2):
                nc.sync.dma_start(
                    xtb[:].rearrange("p (k n) -> p k n", k=KC)[:, 4 * half:4 * half + 4, :],
                    xtb_d.rearrange("(k p) n -> p k n", k=KC)[:, 4 * half:4 * half + 4, :])
            nc.sync.dma_start(maskA[:], maskA_d[:])
            nc.sync.dma_start(maskB[:], maskB_d[:])
            nc.sync.dma_start(trib[:], trib_d[:])
            for k in range(2):
                nc.sync.dma_start(wpb[k][:], wpb_d[128 * k:128 * (k + 1), :])
            nc.vector.memset(onesb[:], 1.0)
            # zero the pad rows of the staging tiles once
            for t in ks8:
                nc.gpsimd.memset(t[64:128, :], 0)
            for t in qs8:
                nc.gpsimd.memset(t[64:128, :], 0)
            for t in ksb:
                nc.gpsimd.memset(t[64:128, :], 0)
            for t in qsb:
                nc.gpsimd.memset(t[64:128, :], 0)

            # ---- stage B: q/k projection, fp8 DoubleRow ----
            # out channels: [q(0:256), k(256:512)]; mf chunks of 128.
            # group A = (mf0, mf2) -> heads 0,1 done first.
            with tc.tile_pool(name="psB", bufs=1, space="PSUM") as psB:
                for gi, mf in enumerate([0, 2, 1, 3]):
                    pss = [psB.tile([128, 512], F32, tag=f"psB{gi % 2}_{nt}",
                                    name=f"psB{mf}_{nt}") for nt in range(NT)]
                    for kp in range(KC // 2):
                        for nt in range(NT):
                            wb = (kp * 8 + mf * 2) * 128
                            nc.tensor.matmul(
                                pss[nt][:],
                                wqk8[:, wb:wb + 256].rearrange(
                                    "p (two m) -> p two m", two=2),
                                xt8_r[:, 2 * kp:2 * kp + 2,
                                      512 * nt:512 * (nt + 1)],
                                start=(kp == 0), stop=(kp == KC // 2 - 1),
                                perf_mode=DoubleRow)
                    for nt in range(NT):
                        ps = pss[nt]
                        # fp8 q'/k' at 64x scale (+64x bias); ACT evacuates
                        nc.scalar.activation(
                            qk8[mf][:, 512 * nt:512 * (nt + 1)], ps[:],
                            Identity, bias=bqk64[mf][:], scale=1.0)
                        # bf16 true-scale for the j0 path (DVE, off ACT)
                        if mf in (0, 1) and nt == 0:
                            nc.vector.tensor_scalar(
                                qkb[mf][:], ps[:], 1.0 / WSCALE, bqkt[mf][:],
                                op0=MULT, op1=ADD)
                        if mf in (2, 3) and nt == 0:
                            nc.vector.tensor_scalar(
                                kkb[mf - 2][:], ps[:], 1.0 / WSCALE, bqkt[mf][:],
                                op0=MULT, op1=ADD)

            # ---- stage C: v projection, bf16 (+ ones col via K=1 bias matmul) ----
            with tc.tile_pool(name="psC", bufs=3, space="PSUM") as psC:
                for mt in range(T // 128):
                    ps = psC.tile([128, VW], F32, tag="psv", name=f"psv{mt}")
                    for kk in range(KC):
                        nc.tensor.matmul(
                            ps[:],
                            xtb[:, T * kk + 128 * mt:T * kk + 128 * (mt + 1)],
                            wvb[:, VW * kk:VW * (kk + 1)],
                            start=(kk == 0), stop=False)
                    nc.tensor.matmul(ps[:], onesb[:, :], bvb[:],
                                     start=False, stop=True)
                    nc.scalar.activation(v8[:, VW * mt:VW * (mt + 1)],
                                             ps[:], Copy)
                    if mt < 4:
                        nc.vector.tensor_copy(vb[:, VW * mt:VW * (mt + 1)],
                                              ps[:])
                    if mt % 2 == 1:
                        pp = mt // 2
                        for hh in range(HPG):
                            for s in range(2):
                                dst = ((pp * 4 + hh) * 2 + s) * 128
                                nc.sync.dma_start(
                                    v8p[:, dst:dst + 128],
                                    v8[:, VW * (2 * pp + s) + 65 * hh:
                                       VW * (2 * pp + s) + 65 * hh + 128])

            # ---- stage D: attention ----
            # pair schedule per head: j0 (bf16) pairs then j>=1 (fp8 DR) pairs
            with (
                tc.tile_pool(name="psA", bufs=2, space="PSUM") as psA,
                tc.tile_pool(name="psAV", bufs=1, space="PSUM") as psAV,
                tc.tile_pool(name="etp", bufs=4) as etp,
                tc.tile_pool(name="et0p", bufs=2) as et0p,
                tc.tile_pool(name="recp", bufs=2) as recp,
                tc.tile_pool(name="bcp", bufs=2) as bcp,
            ):
                act_load = [0.0]   # running col-count per engine for balance
                dve_load = [0.0]

                for h in range(HPG):
                    qtile, off = h // 2, 64 * (h % 2)
                    ktile = 2 + h // 2
                    # --- staging DMAs ---
                    ks = ks8[h % 2]
                    nc.sync.dma_start(ks[0:64, :], qk8[ktile][off:off + 64, :])
                    kb = ksb[h % 2]
                    nc.sync.dma_start(kb[0:64, :], kkb[h // 2][off:off + 64, 0:512])
                    qb = qsb[h % 2]
                    nc.sync.dma_start(qb[0:64, :], qkb[h // 2][off:off + 64, :])
                    qss = {}
                    for j in range(1, NT):
                        t = qs8[(h % 2) * 3 + (j - 1)]
                        nc.sync.dma_start(
                            t[0:64, :],
                            qk8[qtile][off:off + 64, 512 * j:512 * (j + 1)])
                        qss[j] = t

                    avp = {}

                    # pair list: (j, p, c0pair, kind)
                    pairs = []
                    pairs.append((0, 0, 0, "j0"))
                    pairs.append((0, 1, 256, "j0"))
                    for j in range(1, NT):
                        for p in range(2 * j + 2):
                            i0 = 2 * p
                            if i0 == 4 * j:
                                pairs.append((j, p, 0, "dA"))
                            elif i0 == 4 * j + 2:
                                pairs.append((j, p, 256, "dB"))
                            else:
                                pairs.append((j, p, 0, "nd"))

                    pending_av = []

                    def emit_av(rec):
                        j, p, c0p, kind, et = rec
                        if p == 0:
                            avp[j] = psAV.tile([128, 512], F32, tag=f"avj{j}",
                                               name=f"avps{h}_{j}")
                        if kind == "j0":
                            for s in range(2):
                                i = 2 * p + s
                                cc = 128 * i
                                nc.tensor.matmul(
                                    avp[0][:, cc:512],
                                    vb[:, VW * i + 65 * h:VW * i + 65 * h + 128],
                                    et[:, 512 * s + cc:512 * s + 512],
                                    start=(i == 0), stop=(i == 3))
                            if p == 1:
                                normalize(h, 0)
                        else:
                            if c0p == 0:
                                et_r = et[:].bitcast(FP8).rearrange(
                                    "p (two n) -> p two n", two=2)
                            else:
                                et_r = et[:].bitcast(FP8)[
                                    :, 256:768].rearrange(
                                    "p (two n) -> p two n", two=2)
                            vbase = ((p * 4 + h) * 2) * 128
                            nc.tensor.matmul(
                                avp[j][:, c0p:512],
                                v8p[:, vbase:vbase + 256].rearrange(
                                    "p (two m) -> p two m", two=2),
                                et_r,
                                start=(p == 0), stop=(p == 2 * j + 1),
                                perf_mode=DoubleRow)
                            if p == 2 * j + 1:
                                normalize(h, j)

                    def normalize(h_, j_):
                        u = h_ * NT + j_
                        rowb = recp.tile([1, 512], F32, tag="rowb",
                                         name=f"rowb{h_}_{j_}")
                        if (h_ + j_) % 2 == 0:
                            nc.scalar.activation(rowb[:], avp[j_][64:65, :],
                                                 Copy)
                        else:
                            nc.vector.tensor_copy(rowb[:], avp[j_][64:65, :])
                        nc.sync.dma_start(den_d[u:u + 1, :], rowb[:])
                        den2 = recp.tile([128, 4], F32, tag="den2",
                                         name=f"den2_{h_}_{j_}")
                        nc.sync.dma_start(
                            den2[:], bass.AP(den_d.tensor, u * 512,
                                             [[4, 128], [1, 4]]))
                        rec2 = recp.tile([128, 4], F32, tag="rec2",
                                         name=f"rec2_{h_}_{j_}")
                        nc.vector.reciprocal(rec2[:], den2[:])
                        nc.sync.dma_start(
                            bass.AP(rec_d.tensor, u * 512, [[4, 128], [1, 4]]),
                            rec2[:])
                        bc = bcp.tile([64, 512], F32, tag="bc",
                                      name=f"bc{h_}_{j_}")
                        nc.sync.dma_start(
                            bc[:], bass.AP(rec_d.tensor, u * 512,
                                           [[0, 64], [1, 512]]))
                        off_ = 64 * (h_ % 2)
                        nc.vector.scalar_tensor_tensor(
                            outT[h_ // 2][off_:off_ + 64,
                                          512 * j_:512 * (j_ + 1)],
                            avp[j_][0:64, :], 1.0, bc[:],
                            op0=MULT, op1=MULT)

                    LAG = 2
                    for ui, (j, p, c0p, kind) in enumerate(pairs):
                        # attT pair -> psum pair tile [128, 1024]
                        aps = psA.tile([128, 1024], F32, tag="aps",
                                       name=f"aps{h}_{j}_{p}")
                        if kind == "j0":
                            et = et0p.tile([128, 1024], BF16, tag="et0",
                                           name=f"et0_{h}_{p}")
                            for s in range(2):
                                i = 2 * p + s
                                cc = 128 * i
                                nc.tensor.matmul(
                                    aps[:, 512 * s + cc:512 * s + 512],
                                    kb[:, 128 * i:128 * (i + 1)],
                                    qb[:, cc:512], start=True, stop=True)
                                nc.scalar.activation(
                                    et[:, 512 * s + cc:512 * s + 512],
                                    aps[:, 512 * s + cc:512 * s + 512],
                                    Exp, scale=0.125)
                                act_load[0] += 512 - cc
                                # mask diag block via Pool multiply
                                nc.gpsimd.tensor_mul(
                                    et[:, 512 * s + cc:512 * s + cc + 128],
                                    et[:, 512 * s + cc:512 * s + cc + 128],
                                    trib[:])
                        else:
                            et = etp.tile([128, 1024], I8, tag="et",
                                          name=f"et_{h}_{j}_{p}")
                            for s in range(2):
                                i = 2 * p + s
                                c0u = c0p if kind == "nd" else (
                                    128 * ((2 * p + s) % 4))
                                nc.tensor.matmul(
                                    aps[:, 512 * s + c0u:512 * s + 512],
                                    ks[:, 128 * i:128 * (i + 1)],
                                    qss[j][:, c0u:512], start=True, stop=True)
                            if kind == "nd":
                                # whole pair, no mask: cheaper engine does it
                                if act_load[0] <= dve_load[0]:
                                    nc.scalar.activation(
                                        et[:].bitcast(FP8), aps[:], Exp,
                                        scale=A_BYTE * LN2 / 8.0)
                                    act_load[0] += 1024
                                else:
                                    nc.vector.tensor_scalar(
                                        et[:], aps[:], A_BYTE, B_BYTE,
                                        op0=MULT, op1=ADD)
                                    dve_load[0] += 1024
                            elif kind == "dA":
                                nc.vector.scalar_tensor_tensor(
                                    et[:], aps[:], A_BYTE, maskA[:],
                                    op0=MULT, op1=ADD)
                                dve_load[0] += 1024
                            else:  # dB: psum ranges 256-511 & 768-1023,
                                # compacted out to et [256:768)
                                aps_r = aps[:].rearrange(
                                    "p (k n) -> p k n", k=4)[:, 1:4:2, :]
                                et_r2 = et[:, 256:768].rearrange(
                                    "p (k n) -> p k n", k=2)
                                nc.vector.scalar_tensor_tensor(
                                    et_r2, aps_r, A_BYTE,
                                    maskB[:].rearrange(
                                        "p (k n) -> p k n", k=2),
                                    op0=MULT, op1=ADD)
                                dve_load[0] += 512
                        pending_av.append((j, p, c0p, kind, et))
                        if len(pending_av) > LAG:
                            emit_av(pending_av.pop(0))
                    while pending_av:
                        emit_av(pending_av.pop(0))

            # ---- stage E: y^T partial = wp.T @ outT, bf16 ----
            with (
                tc.tile_pool(name="psP", bufs=2, space="PSUM") as psP,
                tc.tile_pool(name="outp", bufs=4) as outp,
            ):
                for mo in range(8):
                    pss = [psP.tile([128, 512], F32, tag=f"psP{nt}",
                                    name=f"psP{mo}_{nt}") for nt in range(NT)]
                    for k in range(2):
                        for nt in range(NT):
                            nc.tensor.matmul(
                                pss[nt][:], wpb[k][:, 128 * mo:128 * (mo + 1)],
                                outT[k][:, 512 * nt:512 * (nt + 1)],
                                start=(k == 0), stop=(k == 1))
                    ot = outp.tile([128, T], BF16, tag="ot",
                                   name=f"ot{mo}")
                    for nt in range(NT):
                        if nt % 2 == 0:
                            nc.vector.tensor_copy(
                                ot[:, 512 * nt:512 * (nt + 1)], pss[nt][:])
                        else:
                            nc.scalar.activation(
                                ot[:, 512 * nt:512 * (nt + 1)], pss[nt][:],
                                Copy)
                    nc.sync.dma_start(yt_d[128 * mo:128 * (mo + 1), :], ot[:])

    nc.compile()
    return nc


def _shard_inputs(x, w_qkv, b_qkv, w_proj):
    e4 = ml_dtypes.float8_e4m3fn
    bf = ml_dtypes.bfloat16
    in_maps = []
    r = np.arange(128)[:, None]
    c = np.arange(128)[None, :]
    tri01 = (c >= r).astype(np.float32)            # keep iff q >= k
    # byte-domain additive masks: keep -> 56.0, masked -> -1e4
    def seg_tri(q0):
        # [128, 128] tri: rows k-local, cols q-local, keep iff (q0+c) >= r
        return np.where(c >= r, B_BYTE, MASKV).astype(np.float32)
    keep = np.full((128, 128), B_BYTE, np.float32)
    full = np.full((128, 128), MASKV, np.float32)
    maskA = np.concatenate(
        [seg_tri(0), keep, keep, keep,            # slot0: tri@0, open
         full, seg_tri(0), keep, keep], axis=1)   # slot1: full@0, tri@128, open
    maskB = np.concatenate(
        [seg_tri(0), keep,                        # slot0 (cols 256-511): tri, open
         full, seg_tri(0)], axis=1)               # slot1: full, tri
    trib = tri01.astype(bf)

    for core in range(N_CORES):
        b, g = divmod(core, HPG)
        qs = slice(CG * g, CG * (g + 1))
        ks = slice(C + CG * g, C + CG * (g + 1))
        vs = slice(2 * C + CG * g, 2 * C + CG * (g + 1))
        # wqk8: [C, 512] = 64 * [Wq; Wk]^T in e4m3 (no attention scale folded;
        # it lives in A_BYTE / the j0 exp scale)
        wqk = np.concatenate([w_qkv[qs] * WSCALE, w_qkv[ks] * WSCALE],
                             axis=0).T
        # paired dual-fp8 layout: [p, kp, mf, s, m] -> [128, 4096]
        wqk = np.ascontiguousarray(
            wqk.reshape(4, 2, 128, 4, 128).transpose(2, 0, 3, 1, 4)
            .reshape(128, 4096))
        bqk64 = (np.concatenate([b_qkv[qs], b_qkv[ks]]) * WSCALE)[:, None]
        bqkt = np.concatenate([b_qkv[qs], b_qkv[ks]])[:, None]
        wv_base = w_qkv[vs].T
        wv = np.zeros((C, VW), np.float32)
        bv = np.zeros((1, VW), np.float32)
        for h in range(HPG):
            wv[:, 65 * h:65 * h + 64] = wv_base[:, 64 * h:64 * h + 64]
            bv[0, 65 * h:65 * h + 64] = b_qkv[vs][64 * h:64 * h + 64]
            bv[0, 65 * h + 64] = 1.0
        xt = np.ascontiguousarray(x[b].T, np.float32)
        in_maps.append({
            "xt8": xt.astype(e4),
            "xtb": xt.astype(bf),
            "wqk8": np.ascontiguousarray(wqk).astype(e4),
            "bqk64": np.ascontiguousarray(bqk64, np.float32),
            "bqkt": np.ascontiguousarray(bqkt, np.float32),
            "wvb": wv.astype(bf),
            "bvb": bv.astype(bf),
            "wpb": np.ascontiguousarray(
                w_proj[:, CG * g:CG * (g + 1)].T).astype(bf),
            "maskA": maskA,
            "maskB": maskB,
            "trib": trib,
        })
    return in_maps


def kernel(x, w_qkv, b_qkv, w_proj, b_proj):
    x = np.asarray(x, np.float32)
    w_qkv = np.asarray(w_qkv, np.float32)
    b_qkv = np.asarray(b_qkv, np.float32)
    w_proj = np.asarray(w_proj, np.float32)
    b_proj = np.asarray(b_proj, np.float32)

    nc = build_nc()
    in_maps = _shard_inputs(x, w_qkv, b_qkv, w_proj)
    if _trace_flag[0]:
        _ensure_ntff_hook()
    res = run_bass_kernel_spmd(nc, in_maps, core_ids=list(range(N_CORES)),
                               trace=_trace_flag[0])
    _last_results[0] = res

    y = np.empty((B, T, C), np.float32)
    for b in range(B):
        acc = np.zeros((C, T), np.float32)
        for g in range(HPG):
            acc += np.asarray(res.results[HPG * b + g]["yt"], np.float32)
        y[b] = acc.T + b_proj[None, :]
    return y


# revision 4
# speedup vs baseline: 1.2875x; 1.0636x over previous
"""Causal self-attention (B=2, T=2048, C=1024, H=16, D=64) on 8 trn2 cores. v2.

Sharding: core c -> batch b = c // 4, head-group g = c % 4 (4 heads each).

Mixed precision design (validated vs reference in numpy, ~1.1e-2 rel):
  q/k projection : fp8e4 DoubleRow matmuls (x, Wq, Wk in e4m3; W scaled x64)
  v projection   : bf16 matmuls (fp8 v fails the early-token error budget)
  scores (attT)  : fp8e4 operands for q-chunks >=1; bf16 for q-chunk 0
  softmax exp    : q-chunk 0 -> ACT real exp -> bf16 P
                   q-chunks >=1 -> DVE/ACT int8 "bit-trick": byte =
                   round(logit*8/ln2 + 56) interpreted as e4m3 == e^z*(1+-4%)
                   (int8 cast saturates; masked cols -> -128 = -0.0 in e4m3)
  P @ V (AV)     : q-chunk 0: bf16; q-chunks >=1: fp8 DoubleRow over k-chunk
                   pairs (P pair tiles [128, 2*512], v pairs strided in one tile)
  denominator    : ones-column in v_aug; DMA psum row 64 -> DRAM, strided
                   reload, DVE reciprocal, partition-broadcast reload
  out projection : bf16, psum evacuated by ACT/DVE alternately -> bf16 DMA out
"""

import os
import sys
import types

for _p in ("/opt/trn_rl_repo", "/root/.axon_site", "/root/.axon_site/_ro/trn_rl_repo"):
    if os.path.isdir(_p) and _p not in sys.path:
        sys.path.append(_p)

import numpy as np
import ml_dtypes

import concourse.bacc as bacc
import concourse.bass as bass
import concourse.mybir as mybir
import concourse.tile as tile
from concourse.bass_utils import run_bass_kernel_spmd

B, T, C = 2, 2048, 1024
H, D = 16, 64
N_CORES = 8
HPG = 4                 # heads per group (per core)
CG = HPG * D            # 256 channels per head-group
NT = T // 512           # 4 q-chunks of 512
KC = C // 128           # 8 contraction tiles over C
VW = HPG * 65 + 64      # v tile width per token chunk

F32 = mybir.dt.float32
BF16 = mybir.dt.bfloat16
FP8 = mybir.dt.float8e4
I8 = mybir.dt.int8

LN2 = float(np.log(2.0))
WSCALE = 64.0                      # fp8 storage scale for Wq/Wk
A_BYTE = 8.0 / (LN2 * (WSCALE * WSCALE * 8.0))   # logit_true = raw/ (64*64*8)
B_BYTE = 56.0                      # e4m3 byte of 1.0
MASKV = -1e4                       # additive mask (saturates int8 to -128 = -0.0)

DoubleRow = mybir.MatmulPerfMode.DoubleRow
Exp = mybir.ActivationFunctionType.Exp
Identity = mybir.ActivationFunctionType.Identity
Copy = mybir.ActivationFunctionType.Copy
MULT = mybir.AluOpType.mult
ADD = mybir.AluOpType.add

_trace_flag = [False]
_last_results = [None]


def _ensure_ntff_hook():
    if "antenv.axon_hooks" in sys.modules:
        return
    try:
        from trn_agent_boot.trn_boot import _ntff_profile_via_ctypes
    except Exception:
        return
    mod = types.ModuleType("antenv.axon_hooks")
    hook = [None]
    mod.set_axon_ntff_profile_hook = lambda h: hook.__setitem__(0, h)
    mod.get_axon_ntff_profile_hook = lambda: hook[0]
    sys.modules["antenv.axon_hooks"] = mod
    so = "/opt/axon/libaxon_pjrt.so"
    if os.path.exists(so):
        mod.set_axon_ntff_profile_hook(_ntff_profile_via_ctypes(so))


def build_nc():
    nc = bacc.Bacc("TRN2", target_bir_lowering=False, debug=False,
                   num_devices=N_CORES)

    xt8_d = nc.dram_tensor("xt8", [C, T], FP8, kind="ExternalInput").ap()
    xtb_d = nc.dram_tensor("xtb", [C, T], BF16, kind="ExternalInput").ap()
    wqk8_d = nc.dram_tensor("wqk8", [128, 32 * 128], FP8, kind="ExternalInput").ap()
    bqk64_d = nc.dram_tensor("bqk64", [2 * CG, 1], F32, kind="ExternalInput").ap()
    bqkt_d = nc.dram_tensor("bqkt", [2 * CG, 1], F32, kind="ExternalInput").ap()
    wvb_d = nc.dram_tensor("wvb", [C, VW], BF16, kind="ExternalInput").ap()
    bvb_d = nc.dram_tensor("bvb", [1, VW], BF16, kind="ExternalInput").ap()
    wpb_d = nc.dram_tensor("wpb", [CG, C], BF16, kind="ExternalInput").ap()
    maskA_d = nc.dram_tensor("maskA", [128, 1024], F32, kind="ExternalInput").ap()
    maskB_d = nc.dram_tensor("maskB", [128, 512], F32, kind="ExternalInput").ap()
    trib_d = nc.dram_tensor("trib", [128, 128], BF16, kind="ExternalInput").ap()
    yt_d = nc.dram_tensor("yt", [C, T], BF16, kind="ExternalOutput").ap()
    den_d = nc.dram_tensor("den_scratch", [HPG * NT, 512], F32).ap()
    rec_d = nc.dram_tensor("rec_scratch", [HPG * NT, 512], F32).ap()

    with tile.TileContext(nc) as tc:
        with tc.tile_pool(name="const", bufs=1) as cp:
            # ---- persistent SBUF residents ----
            xt8 = cp.tile([128, KC * T], FP8, tag="xt8")          # 16 KB/p
            xtb = cp.tile([128, KC * T], BF16, tag="xtb")         # 32 KB/p
            wqk8 = cp.tile([128, KC * 2 * CG], FP8, tag="wqk8")   # 4 KB/p
            bqk64 = [cp.tile([128, 1], F32, tag=f"bqk64_{m}", name=f"bqk64_{m}") for m in range(4)]
            bqkt = [cp.tile([128, 1], F32, tag=f"bqkt_{m}", name=f"bqkt_{m}") for m in range(4)]
            wvb = cp.tile([128, KC * VW], BF16, tag="wvb")        # 5.2 KB/p
            bvb = cp.tile([1, VW], BF16, tag="bvb")
            onesb = cp.tile([1, 128], BF16, tag="onesb")
            wpb = [cp.tile([128, C], BF16, tag=f"wpb{k}", name=f"wpb{k}") for k in range(2)]
            maskA = cp.tile([128, 1024], F32, tag="maskA")
            maskB = cp.tile([128, 512], F32, tag="maskB")
            trib = cp.tile([128, 128], BF16, tag="trib")
            # qk8[m]: m=0: q heads01, 1: q heads23, 2: k heads01, 3: k heads23
            qk8 = [cp.tile([128, T], FP8, tag=f"qk8_{m}", name=f"qk8_{m}") for m in range(4)]
            # bf16 true-scale q/k, q-chunk 0 only (cols 0:512 of q; k all cols)
            qkb = [cp.tile([128, 512], BF16, tag=f"qkb_{m}", name=f"qkb_{m}") for m in range(2)]
            kkb = [cp.tile([128, 512], BF16, tag=f"kkb_{m}", name=f"kkb_{m}") for m in range(2)]
            v8 = cp.tile([128, (T // 128) * VW], FP8, tag="v8")   # 5.1 KB/p
            v8p = cp.tile([128, 8 * 4 * 256], FP8, tag="v8p")    # 8 KB/p
            vb = cp.tile([128, 4 * VW], BF16, tag="vb")           # chunks 0-3
            outT = [cp.tile([128, T], BF16, tag=f"outT{k}", name=f"outT{k}") for k in range(2)]

            # staging tiles (persistent; zero rows written once)
            ks8 = [cp.tile([128, T], FP8, tag=f"ks8_{i}", name=f"ks8_{i}") for i in range(2)]
            qs8 = [cp.tile([128, 512], FP8, tag=f"qs8_{i}", name=f"qs8_{i}") for i in range(6)]
            ksb = [cp.tile([128, 512], BF16, tag=f"ksb_{i}", name=f"ksb_{i}") for i in range(2)]
            qsb = [cp.tile([128, 512], BF16, tag=f"qsb_{i}", name=f"qsb_{i}") for i in range(2)]

            xt8_r = xt8[:].rearrange("p (k n) -> p k n", k=KC)

            # ---- input DMA: interleave wqk8/xt8 pairs (stage B critical path),
            # then everything else ----
            nc.sync.dma_start(wqk8[:], wqk8_d[:])
            for kp in range(KC // 2):
                nc.sync.dma_start(
                    xt8[:].rearrange("p (k n) -> p k n", k=KC)[:, 2 * kp:2 * kp + 2, :],
                    xt8_d.rearrange("(k p) n -> p k n", k=KC)[:, 2 * kp:2 * kp + 2, :])
            for m in range(4):
                nc.sync.dma_start(bqk64[m][:], bqk64_d[128 * m:128 * (m + 1), :])
                nc.sync.dma_start(bqkt[m][:], bqkt_d[128 * m:128 * (m + 1), :])
            nc.sync.dma_start(wvb[:].rearrange("p (k n) -> p k n", k=KC),
                              wvb_d.rearrange("(k p) n -> p k n", k=KC))
            nc.sync.dma_start(bvb[:], bvb_d[:])
            for half in range(2):
                nc.sync.dma_start(
                    xtb[:].rearrange("p (k n) -> p k n", k=KC)[:, 4 * half:4 * half + 4, :],
                    xtb_d.rearrange("(k p) n -> p k n", k=KC)[:, 4 * half:4 * half + 4, :])
            nc.sync.dma_start(maskA[:], maskA_d[:])
            nc.sync.dma_start(maskB[:], maskB_d[:])
            nc.sync.dma_start(trib[:], trib_d[:])
            for k in range(2):
                nc.sync.dma_start(wpb[k][:], wpb_d[128 * k:128 * (k + 1), :])
            nc.vector.memset(onesb[:], 1.0)
            # zero the pad rows of the staging tiles once
            for t in ks8:
                nc.gpsimd.memset(t[64:128, :], 0)
            for t in qs8:
                nc.gpsimd.memset(t[64:128, :], 0)
            for t in ksb:
                nc.gpsimd.memset(t[64:128, :], 0)
            for t in qsb:
                nc.gpsimd.memset(t[64:128, :], 0)

            # ---- stage B: q/k projection, fp8 DoubleRow ----
            # out channels: [q(0:256), k(256:512)]; mf chunks of 128.
            # group A = (mf0, mf2) -> heads 0,1 done first.
            with tc.tile_pool(name="psB", bufs=1, space="PSUM") as psB:
                for gi, mf in enumerate([0, 2, 1, 3]):
                    pss = [psB.tile([128, 512], F32, tag=f"psB{gi % 2}_{nt}",
                                    name=f"psB{mf}_{nt}") for nt in range(NT)]
                    for kp in range(KC // 2):
                        for nt in range(NT):
                            wb = (kp * 8 + mf * 2) * 128
                            nc.tensor.matmul(
                                pss[nt][:],
                                wqk8[:, wb:wb + 256].rearrange(
                                    "p (two m) -> p two m", two=2),
                                xt8_r[:, 2 * kp:2 * kp + 2,
                                      512 * nt:512 * (nt + 1)],
                                start=(kp == 0), stop=(kp == KC // 2 - 1),
                                perf_mode=DoubleRow)
                    for nt in range(NT):
                        ps = pss[nt]
                        # fp8 q'/k' at 64x scale (+64x bias); ACT evacuates
                        nc.scalar.activation(
                            qk8[mf][:, 512 * nt:512 * (nt + 1)], ps[:],
                            Identity, bias=bqk64[mf][:], scale=1.0)
                        # bf16 true-scale for the j0 path (DVE, off ACT)
                        if mf in (0, 1) and nt == 0:
                            nc.vector.tensor_scalar(
                                qkb[mf][:], ps[:], 1.0 / WSCALE, bqkt[mf][:],
                                op0=MULT, op1=ADD)
                        if mf in (2, 3) and nt == 0:
                            nc.vector.tensor_scalar(
                                kkb[mf - 2][:], ps[:], 1.0 / WSCALE, bqkt[mf][:],
                                op0=MULT, op1=ADD)

            # ---- stage C: v projection, bf16 (+ ones col via K=1 bias matmul) ----
            with tc.tile_pool(name="psC", bufs=3, space="PSUM") as psC:
                for mt in range(T // 128):
                    ps = psC.tile([128, VW], F32, tag="psv", name=f"psv{mt}")
                    for kk in range(KC):
                        nc.tensor.matmul(
                            ps[:],
                            xtb[:, T * kk + 128 * mt:T * kk + 128 * (mt + 1)],
                            wvb[:, VW * kk:VW * (kk + 1)],
                            start=(kk == 0), stop=False)
                    nc.tensor.matmul(ps[:], onesb[:, :], bvb[:],
                                     start=False, stop=True)
                    nc.scalar.activation(v8[:, VW * mt:VW * (mt + 1)],
                                             ps[:], Copy)
                    if mt < 4:
                        nc.vector.tensor_copy(vb[:, VW * mt:VW * (mt + 1)],
                                              ps[:])
                    if mt % 2 == 1:
                        pp = mt // 2
                        for hh in range(HPG):
                            for s in range(2):
                                dst = ((pp * 4 + hh) * 2 + s) * 128
                                nc.sync.dma_start(
                                    v8p[:, dst:dst + 128],
                                    v8[:, VW * (2 * pp + s) + 65 * hh:
                                       VW * (2 * pp + s) + 65 * hh + 128])

            # ---- stage D: attention ----
            # pair schedule per head: j0 (bf16) pairs then j>=1 (fp8 DR) pairs
            with (
                tc.tile_pool(name="psA", bufs=4, space="PSUM") as psA,
                tc.tile_pool(name="psAV", bufs=1, space="PSUM") as psAV,
                tc.tile_pool(name="etp", bufs=4) as etp,
                tc.tile_pool(name="et0p", bufs=2) as et0p,
                tc.tile_pool(name="recp", bufs=2) as recp,
                tc.tile_pool(name="bcp", bufs=2) as bcp,
            ):
                act_load = [0.0]   # running col-count per engine for balance
                dve_load = [0.0]

                for h in range(HPG):
                    qtile, off = h // 2, 64 * (h % 2)
                    ktile = 2 + h // 2
                    # --- staging DMAs ---
                    ks = ks8[h % 2]
                    nc.sync.dma_start(ks[0:64, :], qk8[ktile][off:off + 64, :])
                    kb = ksb[h % 2]
                    nc.sync.dma_start(kb[0:64, :], kkb[h // 2][off:off + 64, 0:512])
                    qb = qsb[h % 2]
                    nc.sync.dma_start(qb[0:64, :], qkb[h // 2][off:off + 64, :])
                    qss = {}
                    for j in range(1, NT):
                        t = qs8[(h % 2) * 3 + (j - 1)]
                        nc.sync.dma_start(
                            t[0:64, :],
                            qk8[qtile][off:off + 64, 512 * j:512 * (j + 1)])
                        qss[j] = t

                    avp = {}
                    dve_load[0] += 3500.0   # norm/recip burden this head
                    act_load[0] += 1300.0

                    # pair list: (j, p, c0pair, kind)
                    pairs = []
                    pairs.append((0, 0, 0, "j0"))
                    pairs.append((0, 1, 256, "j0"))
                    for j in range(1, NT):
                        for p in range(2 * j + 2):
                            i0 = 2 * p
                            if i0 == 4 * j:
                                pairs.append((j, p, 0, "dA"))
                            elif i0 == 4 * j + 2:
                                pairs.append((j, p, 256, "dB"))
                            else:
                                pairs.append((j, p, 0, "nd"))

                    pending_av = []

                    def emit_av(rec):
                        j, p, c0p, kind, et = rec
                        if p == 0:
                            avp[j] = psAV.tile([128, 512], F32, tag=f"avj{j}",
                                               name=f"avps{h}_{j}")
                        if kind == "j0":
                            for s in range(2):
                                i = 2 * p + s
                                cc = 128 * i
                                nc.tensor.matmul(
                                    avp[0][:, cc:512],
                                    vb[:, VW * i + 65 * h:VW * i + 65 * h + 128],
                                    et[:, 512 * s + cc:512 * s + 512],
                                    start=(i == 0), stop=(i == 3))
                            if p == 1:
                                normalize(h, 0)
                        else:
                            if c0p == 0:
                                et_r = et[:].bitcast(FP8).rearrange(
                                    "p (two n) -> p two n", two=2)
                            else:
                                et_r = et[:].bitcast(FP8)[
                                    :, 256:768].rearrange(
                                    "p (two n) -> p two n", two=2)
                            vbase = ((p * 4 + h) * 2) * 128
                            nc.tensor.matmul(
                                avp[j][:, c0p:512],
                                v8p[:, vbase:vbase + 256].rearrange(
                                    "p (two m) -> p two m", two=2),
                                et_r,
                                start=(p == 0), stop=(p == 2 * j + 1),
                                perf_mode=DoubleRow)
                            if p == 2 * j + 1:
                                normalize(h, j)

                    def normalize(h_, j_):
                        u = h_ * NT + j_
                        rowb = recp.tile([1, 512], F32, tag="rowb",
                                         name=f"rowb{h_}_{j_}")
                        if (h_ + j_) % 2 == 0:
                            nc.scalar.activation(rowb[:], avp[j_][64:65, :],
                                                 Copy)
                        else:
                            nc.vector.tensor_copy(rowb[:], avp[j_][64:65, :])
                        nc.sync.dma_start(den_d[u:u + 1, :], rowb[:])
                        den2 = recp.tile([128, 4], F32, tag="den2",
                                         name=f"den2_{h_}_{j_}")
                        nc.sync.dma_start(
                            den2[:], bass.AP(den_d.tensor, u * 512,
                                             [[4, 128], [1, 4]]))
                        rec2 = recp.tile([128, 4], F32, tag="rec2",
                                         name=f"rec2_{h_}_{j_}")
                        nc.vector.reciprocal(rec2[:], den2[:])
                        nc.sync.dma_start(
                            bass.AP(rec_d.tensor, u * 512, [[4, 128], [1, 4]]),
                            rec2[:])
                        bc = bcp.tile([64, 512], F32, tag="bc",
                                      name=f"bc{h_}_{j_}")
                        nc.sync.dma_start(
                            bc[:], bass.AP(rec_d.tensor, u * 512,
                                           [[0, 64], [1, 512]]))
                        off_ = 64 * (h_ % 2)
                        nc.vector.scalar_tensor_tensor(
                            outT[h_ // 2][off_:off_ + 64,
                                          512 * j_:512 * (j_ + 1)],
                            avp[j_][0:64, :], 1.0, bc[:],
                            op0=MULT, op1=MULT)

                    LAG = 2
                    for ui, (j, p, c0p, kind) in enumerate(pairs):
                        # attT: per-UNIT 1-bank psum tiles so unit u+4's
                        # alloc waits only exp(u), not the whole pair
                        if kind == "j0":
                            et = et0p.tile([128, 1024], BF16, tag="et0",
                                           name=f"et0_{h}_{p}")
                            for s in range(2):
                                i = 2 * p + s
                                cc = 128 * i
                                aps = psA.tile([128, 512], F32, tag="aps",
                                               name=f"aps{h}_{j}_{p}_{s}")
                                nc.tensor.matmul(
                                    aps[:, cc:512],
                                    kb[:, 128 * i:128 * (i + 1)],
                                    qb[:, cc:512], start=True, stop=True)
                                nc.scalar.activation(
                                    et[:, 512 * s + cc:512 * s + 512],
                                    aps[:, cc:512],
                                    Exp, scale=0.125)
                                act_load[0] += 512 - cc
                                # mask diag block via Pool multiply
                                nc.gpsimd.tensor_mul(
                                    et[:, 512 * s + cc:512 * s + cc + 128],
                                    et[:, 512 * s + cc:512 * s + cc + 128],
                                    trib[:])
                        else:
                            et = etp.tile([128, 1024], I8, tag="et",
                                          name=f"et_{h}_{j}_{p}")
                            for s in range(2):
                                i = 2 * p + s
                                c0u = c0p if kind == "nd" else (
                                    128 * ((2 * p + s) % 4))
                                aps = psA.tile([128, 512], F32, tag="aps",
                                               name=f"aps{h}_{j}_{p}_{s}")
                                nc.tensor.matmul(
                                    aps[:, c0u:512],
                                    ks[:, 128 * i:128 * (i + 1)],
                                    qss[j][:, c0u:512], start=True, stop=True)
                                # exp right after this half's attT
                                if kind == "nd":
                                    if act_load[0] <= dve_load[0]:
                                        nc.scalar.activation(
                                            et[:].bitcast(FP8)[
                                                :, 512 * s:512 * (s + 1)],
                                            aps[:],
                                            Exp, scale=A_BYTE * LN2 / 8.0)
                                        act_load[0] += 512
                                    else:
                                        nc.vector.tensor_scalar(
                                            et[:, 512 * s:512 * (s + 1)],
                                            aps[:],
                                            A_BYTE, B_BYTE,
                                            op0=MULT, op1=ADD)
                                        dve_load[0] += 512
                                elif kind == "dA":
                                    nc.vector.scalar_tensor_tensor(
                                        et[:, 512 * s:512 * (s + 1)],
                                        aps[:],
                                        A_BYTE,
                                        maskA[:, 512 * s:512 * (s + 1)],
                                        op0=MULT, op1=ADD)
                                    dve_load[0] += 512
                                else:  # dB: valid psum cols [256:512); et
                                    # compacted to [256:512)+[512:768)
                                    nc.vector.scalar_tensor_tensor(
                                        et[:, 256 * (s + 1):256 * (s + 2)],
                                        aps[:, 256:512],
                                        A_BYTE,
                                        maskB[:, 256 * s:256 * (s + 1)],
                                        op0=MULT, op1=ADD)
                                    dve_load[0] += 256
                        pending_av.append((j, p, c0p, kind, et))
                        if len(pending_av) > LAG:
                            emit_av(pending_av.pop(0))
                    while pending_av:
                        emit_av(pending_av.pop(0))

            # ---- stage E: y^T partial = wp.T @ outT, bf16 ----
            with (
                tc.tile_pool(name="psP", bufs=2, space="PSUM") as psP,
                tc.tile_pool(name="outp", bufs=4) as outp,
            ):
                for mo in range(8):
                    pss = [psP.tile([128, 512], F32, tag=f"psP{nt}",
                                    name=f"psP{mo}_{nt}") for nt in range(NT)]
                    for k in range(2):
                        for nt in range(NT):
                            nc.tensor.matmul(
                                pss[nt][:], wpb[k][:, 128 * mo:128 * (mo + 1)],
                                outT[k][:, 512 * nt:512 * (nt + 1)],
                                start=(k == 0), stop=(k == 1))
                    ot = outp.tile([128, T], BF16, tag="ot",
                                   name=f"ot{mo}")
                    for nt in range(NT):
                        if nt % 2 == 0:
                            nc.vector.tensor_copy(
                                ot[:, 512 * nt:512 * (nt + 1)], pss[nt][:])
                        else:
                            nc.scalar.activation(
                                ot[:, 512 * nt:512 * (nt + 1)], pss[nt][:],
                                Copy)
                    nc.sync.dma_start(yt_d[128 * mo:128 * (mo + 1), :], ot[:])

    nc.compile()
    return nc


def _shard_inputs(x, w_qkv, b_qkv, w_proj):
    e4 = ml_dtypes.float8_e4m3fn
    bf = ml_dtypes.bfloat16
    in_maps = []
    r = np.arange(128)[:, None]
    c = np.arange(128)[None, :]
    tri01 = (c >= r).astype(np.float32)            # keep iff q >= k
    # byte-domain additive masks: keep -> 56.0, masked -> -1e4
    def seg_tri(q0):
        # [128, 128] tri: rows k-local, cols q-local, keep iff (q0+c) >= r
        return np.where(c >= r, B_BYTE, MASKV).astype(np.float32)
    keep = np.full((128, 128), B_BYTE, np.float32)
    full = np.full((128, 128), MASKV, np.float32)
    maskA = np.concatenate(
        [seg_tri(0), keep, keep, keep,            # slot0: tri@0, open
         full, seg_tri(0), keep, keep], axis=1)   # slot1: full@0, tri@128, open
    maskB = np.concatenate(
        [seg_tri(0), keep,                        # slot0 (cols 256-511): tri, open
         full, seg_tri(0)], axis=1)               # slot1: full, tri
    trib = tri01.astype(bf)

    for core in range(N_CORES):
        b, g = divmod(core, HPG)
        qs = slice(CG * g, CG * (g + 1))
        ks = slice(C + CG * g, C + CG * (g + 1))
        vs = slice(2 * C + CG * g, 2 * C + CG * (g + 1))
        # wqk8: [C, 512] = 64 * [Wq; Wk]^T in e4m3 (no attention scale folded;
        # it lives in A_BYTE / the j0 exp scale)
        wqk = np.concatenate([w_qkv[qs] * WSCALE, w_qkv[ks] * WSCALE],
                             axis=0).T
        # paired dual-fp8 layout: [p, kp, mf, s, m] -> [128, 4096]
        wqk = np.ascontiguousarray(
            wqk.reshape(4, 2, 128, 4, 128).transpose(2, 0, 3, 1, 4)
            .reshape(128, 4096))
        bqk64 = (np.concatenate([b_qkv[qs], b_qkv[ks]]) * WSCALE)[:, None]
        bqkt = np.concatenate([b_qkv[qs], b_qkv[ks]])[:, None]
        wv_base = w_qkv[vs].T
        wv = np.zeros((C, VW), np.float32)
        bv = np.zeros((1, VW), np.float32)
        for h in range(HPG):
            wv[:, 65 * h:65 * h + 64] = wv_base[:, 64 * h:64 * h + 64]
            bv[0, 65 * h:65 * h + 64] = b_qkv[vs][64 * h:64 * h + 64]
            bv[0, 65 * h + 64] = 1.0
        xt = np.ascontiguousarray(x[b].T, np.float32)
        in_maps.append({
            "xt8": xt.astype(e4),
            "xtb": xt.astype(bf),
            "wqk8": np.ascontiguousarray(wqk).astype(e4),
            "bqk64": np.ascontiguousarray(bqk64, np.float32),
            "bqkt": np.ascontiguousarray(bqkt, np.float32),
            "wvb": wv.astype(bf),
            "bvb": bv.astype(bf),
            "wpb": np.ascontiguousarray(
                w_proj[:, CG * g:CG * (g + 1)].T).astype(bf),
            "maskA": maskA,
            "maskB": maskB,
            "trib": trib,
        })
    return in_maps


def kernel(x, w_qkv, b_qkv, w_proj, b_proj):
    x = np.asarray(x, np.float32)
    w_qkv = np.asarray(w_qkv, np.float32)
    b_qkv = np.asarray(b_qkv, np.float32)
    w_proj = np.asarray(w_proj, np.float32)
    b_proj = np.asarray(b_proj, np.float32)

    nc = build_nc()
    in_maps = _shard_inputs(x, w_qkv, b_qkv, w_proj)
    if _trace_flag[0]:
        _ensure_ntff_hook()
    res = run_bass_kernel_spmd(nc, in_maps, core_ids=list(range(N_CORES)),
                               trace=_trace_flag[0])
    _last_results[0] = res

    y = np.empty((B, T, C), np.float32)
    for b in range(B):
        acc = np.zeros((C, T), np.float32)
        for g in range(HPG):
            acc += np.asarray(res.results[HPG * b + g]["yt"], np.float32)
        y[b] = acc.T + b_proj[None, :]
    return y


# revision 5
# speedup vs baseline: 1.3128x; 1.0197x over previous
"""Causal self-attention (B=2, T=2048, C=1024, H=16, D=64) on 8 trn2 cores. v2.

Sharding: core c -> batch b = c // 4, head-group g = c % 4 (4 heads each).

Mixed precision design (validated vs reference in numpy, ~1.1e-2 rel):
  q/k projection : fp8e4 DoubleRow matmuls (x, Wq, Wk in e4m3; W scaled x64)
  v projection   : bf16 matmuls (fp8 v fails the early-token error budget)
  scores (attT)  : fp8e4 operands for q-chunks >=1; bf16 for q-chunk 0
  softmax exp    : q-chunk 0 -> ACT real exp -> bf16 P
                   q-chunks >=1 -> DVE/ACT int8 "bit-trick": byte =
                   round(logit*8/ln2 + 56) interpreted as e4m3 == e^z*(1+-4%)
                   (int8 cast saturates; masked cols -> -128 = -0.0 in e4m3)
  P @ V (AV)     : q-chunk 0: bf16; q-chunks >=1: fp8 DoubleRow over k-chunk
                   pairs (P pair tiles [128, 2*512], v pairs strided in one tile)
  denominator    : ones-column in v_aug; DMA psum row 64 -> DRAM, strided
                   reload, DVE reciprocal, partition-broadcast reload
  out projection : bf16, psum evacuated by ACT/DVE alternately -> bf16 DMA out
"""

import os
import sys
import types

for _p in ("/opt/trn_rl_repo", "/root/.axon_site", "/root/.axon_site/_ro/trn_rl_repo"):
    if os.path.isdir(_p) and _p not in sys.path:
        sys.path.append(_p)

import numpy as np
import ml_dtypes

import concourse.bacc as bacc
import concourse.bass as bass
import concourse.mybir as mybir
import concourse.tile as tile
from concourse.bass_utils import run_bass_kernel_spmd

B, T, C = 2, 2048, 1024
H, D = 16, 64
N_CORES = 8
HPG = 4                 # heads per group (per core)
CG = HPG * D            # 256 channels per head-group
NT = T // 512           # 4 q-chunks of 512
KC = C // 128           # 8 contraction tiles over C
VW = HPG * 65 + 64      # v tile width per token chunk

F32 = mybir.dt.float32
BF16 = mybir.dt.bfloat16
FP8 = mybir.dt.float8e4
I8 = mybir.dt.int8

LN2 = float(np.log(2.0))
WSCALE = 64.0                      # fp8 storage scale for Wq/Wk
A_BYTE = 8.0 / (LN2 * (WSCALE * WSCALE * 8.0))   # logit_true = raw/ (64*64*8)
B_BYTE = 56.0                      # e4m3 byte of 1.0
MASKV = -1e4                       # additive mask (saturates int8 to -128 = -0.0)

DoubleRow = mybir.MatmulPerfMode.DoubleRow
Exp = mybir.ActivationFunctionType.Exp
Identity = mybir.ActivationFunctionType.Identity
Copy = mybir.ActivationFunctionType.Copy
MULT = mybir.AluOpType.mult
ADD = mybir.AluOpType.add

_trace_flag = [False]
_last_results = [None]


def _ensure_ntff_hook():
    if "antenv.axon_hooks" in sys.modules:
        return
    try:
        from trn_agent_boot.trn_boot import _ntff_profile_via_ctypes
    except Exception:
        return
    mod = types.ModuleType("antenv.axon_hooks")
    hook = [None]
    mod.set_axon_ntff_profile_hook = lambda h: hook.__setitem__(0, h)
    mod.get_axon_ntff_profile_hook = lambda: hook[0]
    sys.modules["antenv.axon_hooks"] = mod
    so = "/opt/axon/libaxon_pjrt.so"
    if os.path.exists(so):
        mod.set_axon_ntff_profile_hook(_ntff_profile_via_ctypes(so))


def build_nc():
    nc = bacc.Bacc("TRN2", target_bir_lowering=False, debug=False,
                   num_devices=N_CORES)

    xt8_d = nc.dram_tensor("xt8", [C, T], FP8, kind="ExternalInput").ap()
    xtb_d = nc.dram_tensor("xtb", [C, T], BF16, kind="ExternalInput").ap()
    wqk8_d = nc.dram_tensor("wqk8", [128, 32 * 128], FP8, kind="ExternalInput").ap()
    bqk64_d = nc.dram_tensor("bqk64", [2 * CG, 1], F32, kind="ExternalInput").ap()
    bqkt_d = nc.dram_tensor("bqkt", [2 * CG, 1], F32, kind="ExternalInput").ap()
    wvb_d = nc.dram_tensor("wvb", [C, VW], BF16, kind="ExternalInput").ap()
    bvb_d = nc.dram_tensor("bvb", [1, VW], BF16, kind="ExternalInput").ap()
    wpb_d = nc.dram_tensor("wpb", [CG, C], BF16, kind="ExternalInput").ap()
    maskA_d = nc.dram_tensor("maskA", [128, 1024], F32, kind="ExternalInput").ap()
    maskB_d = nc.dram_tensor("maskB", [128, 512], F32, kind="ExternalInput").ap()
    trib_d = nc.dram_tensor("trib", [128, 128], BF16, kind="ExternalInput").ap()
    yt_d = nc.dram_tensor("yt", [C, T], BF16, kind="ExternalOutput").ap()
    den_d = nc.dram_tensor("den_scratch", [HPG * NT, 512], F32).ap()
    rec_d = nc.dram_tensor("rec_scratch", [HPG * NT, 512], F32).ap()

    with tile.TileContext(nc) as tc:
        with tc.tile_pool(name="const", bufs=1) as cp:
            # ---- persistent SBUF residents ----
            xt8 = cp.tile([128, KC * T], FP8, tag="xt8")          # 16 KB/p
            xtb = cp.tile([128, KC * T], BF16, tag="xtb")         # 32 KB/p
            wqk8 = cp.tile([128, KC * 2 * CG], FP8, tag="wqk8")   # 4 KB/p
            bqk64 = [cp.tile([128, 1], F32, tag=f"bqk64_{m}", name=f"bqk64_{m}") for m in range(4)]
            bqkt = [cp.tile([128, 1], F32, tag=f"bqkt_{m}", name=f"bqkt_{m}") for m in range(4)]
            wvb = cp.tile([128, KC * VW], BF16, tag="wvb")        # 5.2 KB/p
            bvb = cp.tile([1, VW], BF16, tag="bvb")
            onesb = cp.tile([1, 128], BF16, tag="onesb")
            wpb = [cp.tile([128, C], BF16, tag=f"wpb{k}", name=f"wpb{k}") for k in range(2)]
            maskA = cp.tile([128, 1024], F32, tag="maskA")
            maskB = cp.tile([128, 512], F32, tag="maskB")
            trib = cp.tile([128, 128], BF16, tag="trib")
            # qk8[m]: m=0: q heads01, 1: q heads23, 2: k heads01, 3: k heads23
            qk8 = [cp.tile([128, T], FP8, tag=f"qk8_{m}", name=f"qk8_{m}") for m in range(4)]
            # bf16 true-scale q/k, q-chunk 0 only (cols 0:512 of q; k all cols)
            qkb = [cp.tile([128, 512], BF16, tag=f"qkb_{m}", name=f"qkb_{m}") for m in range(2)]
            kkb = [cp.tile([128, 512], BF16, tag=f"kkb_{m}", name=f"kkb_{m}") for m in range(2)]
            v8 = cp.tile([128, (T // 128) * VW], FP8, tag="v8")   # 5.1 KB/p
            v8p = cp.tile([128, 8 * 4 * 256], FP8, tag="v8p")    # 8 KB/p
            vb = cp.tile([128, 4 * VW], BF16, tag="vb")           # chunks 0-3
            outT = [cp.tile([128, T], BF16, tag=f"outT{k}", name=f"outT{k}") for k in range(2)]

            # staging tiles (persistent; zero rows written once)
            ks8 = [cp.tile([128, T], FP8, tag=f"ks8_{i}", name=f"ks8_{i}") for i in range(2)]
            qs8 = [cp.tile([128, 512], FP8, tag=f"qs8_{i}", name=f"qs8_{i}") for i in range(6)]
            ksb = [cp.tile([128, 512], BF16, tag=f"ksb_{i}", name=f"ksb_{i}") for i in range(2)]
            qsb = [cp.tile([128, 512], BF16, tag=f"qsb_{i}", name=f"qsb_{i}") for i in range(2)]

            xt8_r = xt8[:].rearrange("p (k n) -> p k n", k=KC)

            # ---- input DMA: interleave wqk8/xt8 pairs (stage B critical path),
            # then everything else ----
            nc.sync.dma_start(wqk8[:], wqk8_d[:])
            for kp in range(KC // 2):
                nc.sync.dma_start(
                    xt8[:].rearrange("p (k n) -> p k n", k=KC)[:, 2 * kp:2 * kp + 2, :],
                    xt8_d.rearrange("(k p) n -> p k n", k=KC)[:, 2 * kp:2 * kp + 2, :])
            for m in range(4):
                nc.sync.dma_start(bqk64[m][:], bqk64_d[128 * m:128 * (m + 1), :])
                nc.sync.dma_start(bqkt[m][:], bqkt_d[128 * m:128 * (m + 1), :])
            nc.sync.dma_start(wvb[:].rearrange("p (k n) -> p k n", k=KC),
                              wvb_d.rearrange("(k p) n -> p k n", k=KC))
            nc.sync.dma_start(bvb[:], bvb_d[:])
            for half in range(2):
                nc.sync.dma_start(
                    xtb[:].rearrange("p (k n) -> p k n", k=KC)[:, 4 * half:4 * half + 4, :],
                    xtb_d.rearrange("(k p) n -> p k n", k=KC)[:, 4 * half:4 * half + 4, :])
            nc.sync.dma_start(maskA[:], maskA_d[:])
            nc.sync.dma_start(maskB[:], maskB_d[:])
            nc.sync.dma_start(trib[:], trib_d[:])
            for k in range(2):
                nc.sync.dma_start(wpb[k][:], wpb_d[128 * k:128 * (k + 1), :])
            nc.vector.memset(onesb[:], 1.0)
            # zero the pad rows of the staging tiles once
            for t in ks8:
                nc.gpsimd.memset(t[64:128, :], 0)
            for t in qs8:
                nc.gpsimd.memset(t[64:128, :], 0)
            for t in ksb:
                nc.gpsimd.memset(t[64:128, :], 0)
            for t in qsb:
                nc.gpsimd.memset(t[64:128, :], 0)

            # ---- stage B: q/k projection, fp8 DoubleRow ----
            # out channels: [q(0:256), k(256:512)]; mf chunks of 128.
            # group A = (mf0, mf2) -> heads 0,1 done first.
            with tc.tile_pool(name="psB", bufs=1, space="PSUM") as psB:
                for gi, mf in enumerate([0, 2, 1, 3]):
                    pss = [psB.tile([128, 512], F32, tag=f"psB{gi % 2}_{nt}",
                                    name=f"psB{mf}_{nt}") for nt in range(NT)]
                    for kp in range(KC // 2):
                        for nt in range(NT):
                            wb = (kp * 8 + mf * 2) * 128
                            nc.tensor.matmul(
                                pss[nt][:],
                                wqk8[:, wb:wb + 256].rearrange(
                                    "p (two m) -> p two m", two=2),
                                xt8_r[:, 2 * kp:2 * kp + 2,
                                      512 * nt:512 * (nt + 1)],
                                start=(kp == 0), stop=(kp == KC // 2 - 1),
                                perf_mode=DoubleRow)
                    for nt in range(NT):
                        ps = pss[nt]
                        # fp8 q'/k' at 64x scale (+64x bias); ACT evacuates
                        nc.scalar.activation(
                            qk8[mf][:, 512 * nt:512 * (nt + 1)], ps[:],
                            Identity, bias=bqk64[mf][:], scale=1.0)
                        # bf16 true-scale for the j0 path (DVE, off ACT)
                        if mf in (0, 1) and nt == 0:
                            nc.vector.tensor_scalar(
                                qkb[mf][:], ps[:], 1.0 / WSCALE, bqkt[mf][:],
                                op0=MULT, op1=ADD)
                        if mf in (2, 3) and nt == 0:
                            nc.vector.tensor_scalar(
                                kkb[mf - 2][:], ps[:], 1.0 / WSCALE, bqkt[mf][:],
                                op0=MULT, op1=ADD)

            # ---- stage C: v projection, bf16 (+ ones col via K=1 bias matmul) ----
            with tc.tile_pool(name="psC", bufs=3, space="PSUM") as psC:
                for mt in range(T // 128):
                    ps = psC.tile([128, VW], F32, tag="psv", name=f"psv{mt}")
                    for kk in range(KC):
                        nc.tensor.matmul(
                            ps[:],
                            xtb[:, T * kk + 128 * mt:T * kk + 128 * (mt + 1)],
                            wvb[:, VW * kk:VW * (kk + 1)],
                            start=(kk == 0), stop=False)
                    nc.tensor.matmul(ps[:], onesb[:, :], bvb[:],
                                     start=False, stop=True)
                    nc.scalar.activation(v8[:, VW * mt:VW * (mt + 1)],
                                             ps[:], Copy)
                    if mt < 4:
                        nc.vector.tensor_copy(vb[:, VW * mt:VW * (mt + 1)],
                                              ps[:])
                    if mt % 2 == 1:
                        pp = mt // 2
                        for hh in range(HPG):
                            for s in range(2):
                                dst = ((pp * 4 + hh) * 2 + s) * 128
                                nc.sync.dma_start(
                                    v8p[:, dst:dst + 128],
                                    v8[:, VW * (2 * pp + s) + 65 * hh:
                                       VW * (2 * pp + s) + 65 * hh + 128])

            # ---- stage D: attention ----
            # pair schedule per head: j0 (bf16) pairs then j>=1 (fp8 DR) pairs
            with (
                tc.tile_pool(name="psA", bufs=4, space="PSUM") as psA,
                tc.tile_pool(name="psAV", bufs=1, space="PSUM") as psAV,
                tc.tile_pool(name="etp", bufs=5) as etp,
                tc.tile_pool(name="et0p", bufs=3) as et0p,
                tc.tile_pool(name="recp", bufs=2) as recp,
                tc.tile_pool(name="bcp", bufs=2) as bcp,
            ):
                act_load = [0.0]   # running col-count per engine for balance
                dve_load = [0.0]

                for h in range(HPG):
                    qtile, off = h // 2, 64 * (h % 2)
                    ktile = 2 + h // 2
                    # --- staging DMAs ---
                    ks = ks8[h % 2]
                    nc.sync.dma_start(ks[0:64, :], qk8[ktile][off:off + 64, :])
                    kb = ksb[h % 2]
                    nc.sync.dma_start(kb[0:64, :], kkb[h // 2][off:off + 64, 0:512])
                    qb = qsb[h % 2]
                    nc.sync.dma_start(qb[0:64, :], qkb[h // 2][off:off + 64, :])
                    qss = {}
                    for j in range(1, NT):
                        t = qs8[(h % 2) * 3 + (j - 1)]
                        nc.sync.dma_start(
                            t[0:64, :],
                            qk8[qtile][off:off + 64, 512 * j:512 * (j + 1)])
                        qss[j] = t

                    avp = {}
                    dve_load[0] += 3500.0   # norm/recip burden this head
                    act_load[0] += 1300.0

                    # pair list: (j, p, c0pair, kind)
                    pairs = []
                    pairs.append((0, 0, 0, "j0"))
                    pairs.append((0, 1, 256, "j0"))
                    for j in range(1, NT):
                        for p in range(2 * j + 2):
                            i0 = 2 * p
                            if i0 == 4 * j:
                                pairs.append((j, p, 0, "dA"))
                            elif i0 == 4 * j + 2:
                                pairs.append((j, p, 256, "dB"))
                            else:
                                pairs.append((j, p, 0, "nd"))

                    pending_av = []

                    def emit_av(rec):
                        j, p, c0p, kind, et = rec
                        if p == 0:
                            avp[j] = psAV.tile([128, 512], F32, tag=f"avj{j}",
                                               name=f"avps{h}_{j}")
                        if kind == "j0":
                            for s in range(2):
                                i = 2 * p + s
                                cc = 128 * i
                                nc.tensor.matmul(
                                    avp[0][:, cc:512],
                                    vb[:, VW * i + 65 * h:VW * i + 65 * h + 128],
                                    et[:, 512 * s + cc:512 * s + 512],
                                    start=(i == 0), stop=(i == 3))
                            if p == 1:
                                normalize(h, 0)
                        else:
                            if c0p == 0:
                                et_r = et[:].bitcast(FP8).rearrange(
                                    "p (two n) -> p two n", two=2)
                            else:
                                et_r = et[:].bitcast(FP8)[
                                    :, 256:768].rearrange(
                                    "p (two n) -> p two n", two=2)
                            vbase = ((p * 4 + h) * 2) * 128
                            nc.tensor.matmul(
                                avp[j][:, c0p:512],
                                v8p[:, vbase:vbase + 256].rearrange(
                                    "p (two m) -> p two m", two=2),
                                et_r,
                                start=(p == 0), stop=(p == 2 * j + 1),
                                perf_mode=DoubleRow)
                            if p == 2 * j + 1:
                                normalize(h, j)

                    def normalize(h_, j_):
                        u = h_ * NT + j_
                        rowb = recp.tile([1, 512], F32, tag="rowb",
                                         name=f"rowb{h_}_{j_}")
                        if (h_ + j_) % 2 == 0:
                            nc.scalar.activation(rowb[:], avp[j_][64:65, :],
                                                 Copy)
                        else:
                            nc.vector.tensor_copy(rowb[:], avp[j_][64:65, :])
                        nc.sync.dma_start(den_d[u:u + 1, :], rowb[:])
                        den2 = recp.tile([128, 4], F32, tag="den2",
                                         name=f"den2_{h_}_{j_}")
                        nc.sync.dma_start(
                            den2[:], bass.AP(den_d.tensor, u * 512,
                                             [[4, 128], [1, 4]]))
                        rec2 = recp.tile([128, 4], F32, tag="rec2",
                                         name=f"rec2_{h_}_{j_}")
                        nc.vector.reciprocal(rec2[:], den2[:])
                        nc.sync.dma_start(
                            bass.AP(rec_d.tensor, u * 512, [[4, 128], [1, 4]]),
                            rec2[:])
                        bc = bcp.tile([64, 512], F32, tag="bc",
                                      name=f"bc{h_}_{j_}")
                        nc.sync.dma_start(
                            bc[:], bass.AP(rec_d.tensor, u * 512,
                                           [[0, 64], [1, 512]]))
                        off_ = 64 * (h_ % 2)
                        nc.vector.scalar_tensor_tensor(
                            outT[h_ // 2][off_:off_ + 64,
                                          512 * j_:512 * (j_ + 1)],
                            avp[j_][0:64, :], 1.0, bc[:],
                            op0=MULT, op1=MULT)

                    LAG = 3
                    for ui, (j, p, c0p, kind) in enumerate(pairs):
                        # attT: per-UNIT 1-bank psum tiles so unit u+4's
                        # alloc waits only exp(u), not the whole pair
                        if kind == "j0":
                            et = et0p.tile([128, 1024], BF16, tag="et0",
                                           name=f"et0_{h}_{p}")
                            for s in range(2):
                                i = 2 * p + s
                                cc = 128 * i
                                aps = psA.tile([128, 512], F32, tag="aps",
                                               name=f"aps{h}_{j}_{p}_{s}")
                                nc.tensor.matmul(
                                    aps[:, cc:512],
                                    kb[:, 128 * i:128 * (i + 1)],
                                    qb[:, cc:512], start=True, stop=True)
                                nc.scalar.activation(
                                    et[:, 512 * s + cc:512 * s + 512],
                                    aps[:, cc:512],
                                    Exp, scale=0.125)
                                act_load[0] += 512 - cc
                                # mask diag block via Pool multiply
                                nc.gpsimd.tensor_mul(
                                    et[:, 512 * s + cc:512 * s + cc + 128],
                                    et[:, 512 * s + cc:512 * s + cc + 128],
                                    trib[:])
                        else:
                            et = etp.tile([128, 1024], I8, tag="et",
                                          name=f"et_{h}_{j}_{p}")
                            for s in range(2):
                                i = 2 * p + s
                                c0u = c0p if kind == "nd" else (
                                    128 * ((2 * p + s) % 4))
                                aps = psA.tile([128, 512], F32, tag="aps",
                                               name=f"aps{h}_{j}_{p}_{s}")
                                nc.tensor.matmul(
                                    aps[:, c0u:512],
                                    ks[:, 128 * i:128 * (i + 1)],
                                    qss[j][:, c0u:512], start=True, stop=True)
                                # exp right after this half's attT
                                if kind == "nd":
                                    if act_load[0] <= dve_load[0]:
                                        nc.scalar.activation(
                                            et[:].bitcast(FP8)[
                                                :, 512 * s:512 * (s + 1)],
                                            aps[:],
                                            Exp, scale=A_BYTE * LN2 / 8.0)
                                        act_load[0] += 512
                                    else:
                                        nc.vector.tensor_scalar(
                                            et[:, 512 * s:512 * (s + 1)],
                                            aps[:],
                                            A_BYTE, B_BYTE,
                                            op0=MULT, op1=ADD)
                                        dve_load[0] += 512
                                elif kind == "dA":
                                    nc.vector.scalar_tensor_tensor(
                                        et[:, 512 * s:512 * (s + 1)],
                                        aps[:],
                                        A_BYTE,
                                        maskA[:, 512 * s:512 * (s + 1)],
                                        op0=MULT, op1=ADD)
                                    dve_load[0] += 512
                                else:  # dB: valid psum cols [256:512); et
                                    # compacted to [256:512)+[512:768)
                                    nc.vector.scalar_tensor_tensor(
                                        et[:, 256 * (s + 1):256 * (s + 2)],
                                        aps[:, 256:512],
                                        A_BYTE,
                                        maskB[:, 256 * s:256 * (s + 1)],
                                        op0=MULT, op1=ADD)
                                    dve_load[0] += 256
                        pending_av.append((j, p, c0p, kind, et))
                        if len(pending_av) > LAG:
                            emit_av(pending_av.pop(0))
                    while pending_av:
                        emit_av(pending_av.pop(0))

            # ---- stage E: y^T partial = wp.T @ outT, bf16 ----
            with (
                tc.tile_pool(name="psP", bufs=2, space="PSUM") as psP,
                tc.tile_pool(name="outp", bufs=4) as outp,
            ):
                for mo in range(8):
                    pss = [psP.tile([128, 512], F32, tag=f"psP{nt}",
                                    name=f"psP{mo}_{nt}") for nt in range(NT)]
                    for k in range(2):
                        for nt in range(NT):
                            nc.tensor.matmul(
                                pss[nt][:], wpb[k][:, 128 * mo:128 * (mo + 1)],
                                outT[k][:, 512 * nt:512 * (nt + 1)],
                                start=(k == 0), stop=(k == 1))
                    ot = outp.tile([128, T], BF16, tag="ot",
                                   name=f"ot{mo}")
                    for nt in range(NT):
                        if nt % 2 == 0:
                            nc.vector.tensor_copy(
                                ot[:, 512 * nt:512 * (nt + 1)], pss[nt][:])
                        else:
                            nc.scalar.activation(
                                ot[:, 512 * nt:512 * (nt + 1)], pss[nt][:],
                                Copy)
                    nc.sync.dma_start(yt_d[128 * mo:128 * (mo + 1), :], ot[:])

    nc.compile()
    return nc


def _shard_inputs(x, w_qkv, b_qkv, w_proj):
    e4 = ml_dtypes.float8_e4m3fn
    bf = ml_dtypes.bfloat16
    in_maps = []
    r = np.arange(128)[:, None]
    c = np.arange(128)[None, :]
    tri01 = (c >= r).astype(np.float32)            # keep iff q >= k
    # byte-domain additive masks: keep -> 56.0, masked -> -1e4
    def seg_tri(q0):
        # [128, 128] tri: rows k-local, cols q-local, keep iff (q0+c) >= r
        return np.where(c >= r, B_BYTE, MASKV).astype(np.float32)
    keep = np.full((128, 128), B_BYTE, np.float32)
    full = np.full((128, 128), MASKV, np.float32)
    maskA = np.concatenate(
        [seg_tri(0), keep, keep, keep,            # slot0: tri@0, open
         full, seg_tri(0), keep, keep], axis=1)   # slot1: full@0, tri@128, open
    maskB = np.concatenate(
        [seg_tri(0), keep,                        # slot0 (cols 256-511): tri, open
         full, seg_tri(0)], axis=1)               # slot1: full, tri
    trib = tri01.astype(bf)

    for core in range(N_CORES):
        b, g = divmod(core, HPG)
        qs = slice(CG * g, CG * (g + 1))
        ks = slice(C + CG * g, C + CG * (g + 1))
        vs = slice(2 * C + CG * g, 2 * C + CG * (g + 1))
        # wqk8: [C, 512] = 64 * [Wq; Wk]^T in e4m3 (no attention scale folded;
        # it lives in A_BYTE / the j0 exp scale)
        wqk = np.concatenate([w_qkv[qs] * WSCALE, w_qkv[ks] * WSCALE],
                             axis=0).T
        # paired dual-fp8 layout: [p, kp, mf, s, m] -> [128, 4096]
        wqk = np.ascontiguousarray(
            wqk.reshape(4, 2, 128, 4, 128).transpose(2, 0, 3, 1, 4)
            .reshape(128, 4096))
        bqk64 = (np.concatenate([b_qkv[qs], b_qkv[ks]]) * WSCALE)[:, None]
        bqkt = np.concatenate([b_qkv[qs], b_qkv[ks]])[:, None]
        wv_base = w_qkv[vs].T
        wv = np.zeros((C, VW), np.float32)
        bv = np.zeros((1, VW), np.float32)
        for h in range(HPG):
            wv[:, 65 * h:65 * h + 64] = wv_base[:, 64 * h:64 * h + 64]
            bv[0, 65 * h:65 * h + 64] = b_qkv[vs][64 * h:64 * h + 64]
            bv[0, 65 * h + 64] = 1.0
        xt = np.ascontiguousarray(x[b].T, np.float32)
        in_maps.append({
            "xt8": xt.astype(e4),
            "xtb": xt.astype(bf),
            "wqk8": np.ascontiguousarray(wqk).astype(e4),
            "bqk64": np.ascontiguousarray(bqk64, np.float32),
            "bqkt": np.ascontiguousarray(bqkt, np.float32),
            "wvb": wv.astype(bf),
            "bvb": bv.astype(bf),
            "wpb": np.ascontiguousarray(
                w_proj[:, CG * g:CG * (g + 1)].T).astype(bf),
            "maskA": maskA,
            "maskB": maskB,
            "trib": trib,
        })
    return in_maps


def kernel(x, w_qkv, b_qkv, w_proj, b_proj):
    x = np.asarray(x, np.float32)
    w_qkv = np.asarray(w_qkv, np.float32)
    b_qkv = np.asarray(b_qkv, np.float32)
    w_proj = np.asarray(w_proj, np.float32)
    b_proj = np.asarray(b_proj, np.float32)

    nc = build_nc()
    in_maps = _shard_inputs(x, w_qkv, b_qkv, w_proj)
    if _trace_flag[0]:
        _ensure_ntff_hook()
    res = run_bass_kernel_spmd(nc, in_maps, core_ids=list(range(N_CORES)),
                               trace=_trace_flag[0])
    _last_results[0] = res

    y = np.empty((B, T, C), np.float32)
    for b in range(B):
        acc = np.zeros((C, T), np.float32)
        for g in range(HPG):
            acc += np.asarray(res.results[HPG * b + g]["yt"], np.float32)
        y[b] = acc.T + b_proj[None, :]
    return y


# revision 6
# speedup vs baseline: 1.3374x; 1.0187x over previous
"""Causal self-attention (B=2, T=2048, C=1024, H=16, D=64) on 8 trn2 cores. v2.

Sharding: core c -> batch b = c // 4, head-group g = c % 4 (4 heads each).

Mixed precision design (validated vs reference in numpy, ~1.1e-2 rel):
  q/k projection : fp8e4 DoubleRow matmuls (x, Wq, Wk in e4m3; W scaled x64)
  v projection   : bf16 matmuls (fp8 v fails the early-token error budget)
  scores (attT)  : fp8e4 operands for q-chunks >=1; bf16 for q-chunk 0
  softmax exp    : q-chunk 0 -> ACT real exp -> bf16 P
                   q-chunks >=1 -> DVE/ACT int8 "bit-trick": byte =
                   round(logit*8/ln2 + 56) interpreted as e4m3 == e^z*(1+-4%)
                   (int8 cast saturates; masked cols -> -128 = -0.0 in e4m3)
  P @ V (AV)     : q-chunk 0: bf16; q-chunks >=1: fp8 DoubleRow over k-chunk
                   pairs (P pair tiles [128, 2*512], v pairs strided in one tile)
  denominator    : ones-column in v_aug; DMA psum row 64 -> DRAM, strided
                   reload, DVE reciprocal, partition-broadcast reload
  out projection : bf16, psum evacuated by ACT/DVE alternately -> bf16 DMA out
"""

import os
import sys
import types

for _p in ("/opt/trn_rl_repo", "/root/.axon_site", "/root/.axon_site/_ro/trn_rl_repo"):
    if os.path.isdir(_p) and _p not in sys.path:
        sys.path.append(_p)

import numpy as np
import ml_dtypes

import concourse.bacc as bacc
import concourse.bass as bass
import concourse.mybir as mybir
import concourse.tile as tile
from concourse.bass_utils import run_bass_kernel_spmd

B, T, C = 2, 2048, 1024
H, D = 16, 64
N_CORES = 8
HPG = 4                 # heads per group (per core)
CG = HPG * D            # 256 channels per head-group
NT = T // 512           # 4 q-chunks of 512
KC = C // 128           # 8 contraction tiles over C
VW = HPG * 65 + 64      # v tile width per token chunk

F32 = mybir.dt.float32
BF16 = mybir.dt.bfloat16
FP8 = mybir.dt.float8e4
I8 = mybir.dt.int8

LN2 = float(np.log(2.0))
WSCALE = 64.0                      # fp8 storage scale for Wq/Wk
A_BYTE = 8.0 / (LN2 * (WSCALE * WSCALE * 8.0))   # logit_true = raw/ (64*64*8)
B_BYTE = 56.0                      # e4m3 byte of 1.0
MASKV = -1e4                       # additive mask (saturates int8 to -128 = -0.0)

DoubleRow = mybir.MatmulPerfMode.DoubleRow
Exp = mybir.ActivationFunctionType.Exp
Identity = mybir.ActivationFunctionType.Identity
Copy = mybir.ActivationFunctionType.Copy
MULT = mybir.AluOpType.mult
ADD = mybir.AluOpType.add

_trace_flag = [False]
_last_results = [None]


def _ensure_ntff_hook():
    if "antenv.axon_hooks" in sys.modules:
        return
    try:
        from trn_agent_boot.trn_boot import _ntff_profile_via_ctypes
    except Exception:
        return
    mod = types.ModuleType("antenv.axon_hooks")
    hook = [None]
    mod.set_axon_ntff_profile_hook = lambda h: hook.__setitem__(0, h)
    mod.get_axon_ntff_profile_hook = lambda: hook[0]
    sys.modules["antenv.axon_hooks"] = mod
    so = "/opt/axon/libaxon_pjrt.so"
    if os.path.exists(so):
        mod.set_axon_ntff_profile_hook(_ntff_profile_via_ctypes(so))


def build_nc():
    nc = bacc.Bacc("TRN2", target_bir_lowering=False, debug=False,
                   num_devices=N_CORES)

    xt8_d = nc.dram_tensor("xt8", [C, T], FP8, kind="ExternalInput").ap()
    xtb_d = nc.dram_tensor("xtb", [C, T], BF16, kind="ExternalInput").ap()
    wqk8_d = nc.dram_tensor("wqk8", [128, 32 * 128], FP8, kind="ExternalInput").ap()
    bqk64_d = nc.dram_tensor("bqk64", [2 * CG, 1], F32, kind="ExternalInput").ap()
    bqkt_d = nc.dram_tensor("bqkt", [2 * CG, 1], F32, kind="ExternalInput").ap()
    wvb_d = nc.dram_tensor("wvb", [C, VW], BF16, kind="ExternalInput").ap()
    bvb_d = nc.dram_tensor("bvb", [1, VW], BF16, kind="ExternalInput").ap()
    wpb_d = nc.dram_tensor("wpb", [CG, C], BF16, kind="ExternalInput").ap()
    maskA_d = nc.dram_tensor("maskA", [128, 1024], F32, kind="ExternalInput").ap()
    maskB_d = nc.dram_tensor("maskB", [128, 512], F32, kind="ExternalInput").ap()
    trib_d = nc.dram_tensor("trib", [128, 128], BF16, kind="ExternalInput").ap()
    yt_d = nc.dram_tensor("yt", [C, T], BF16, kind="ExternalOutput").ap()
    den_d = nc.dram_tensor("den_scratch", [HPG * NT, 512], F32).ap()
    rec_d = nc.dram_tensor("rec_scratch", [HPG * NT, 512], F32).ap()

    with tile.TileContext(nc) as tc:
        with tc.tile_pool(name="const", bufs=1) as cp:
            # ---- persistent SBUF residents ----
            xt8 = cp.tile([128, KC * T], FP8, tag="xt8")          # 16 KB/p
            xtb = cp.tile([128, KC * T], BF16, tag="xtb")         # 32 KB/p
            wqk8 = cp.tile([128, KC * 2 * CG], FP8, tag="wqk8")   # 4 KB/p
            bqk64 = [cp.tile([128, 1], F32, tag=f"bqk64_{m}", name=f"bqk64_{m}") for m in range(4)]
            bqkt = [cp.tile([128, 1], F32, tag=f"bqkt_{m}", name=f"bqkt_{m}") for m in range(4)]
            wvb = cp.tile([128, KC * VW], BF16, tag="wvb")        # 5.2 KB/p
            bvb = cp.tile([1, VW], BF16, tag="bvb")
            onesb = cp.tile([1, 128], BF16, tag="onesb")
            wpb = [cp.tile([128, C], BF16, tag=f"wpb{k}", name=f"wpb{k}") for k in range(2)]
            maskA = cp.tile([128, 1024], F32, tag="maskA")
            maskB = cp.tile([128, 512], F32, tag="maskB")
            trib = cp.tile([128, 128], BF16, tag="trib")
            # qk8[m]: m=0: q heads01, 1: q heads23, 2: k heads01, 3: k heads23
            qk8 = [cp.tile([128, T], FP8, tag=f"qk8_{m}", name=f"qk8_{m}") for m in range(4)]
            # bf16 true-scale q/k, q-chunk 0 only (cols 0:512 of q; k all cols)
            qkb = [cp.tile([128, 512], BF16, tag=f"qkb_{m}", name=f"qkb_{m}") for m in range(2)]
            kkb = [cp.tile([128, 512], BF16, tag=f"kkb_{m}", name=f"kkb_{m}") for m in range(2)]
            v8 = cp.tile([128, (T // 128) * VW], FP8, tag="v8")   # 5.1 KB/p
            v8p = cp.tile([128, 8 * 4 * 256], FP8, tag="v8p")    # 8 KB/p
            vb = cp.tile([128, 4 * VW], BF16, tag="vb")           # chunks 0-3
            outT = [cp.tile([128, T], BF16, tag=f"outT{k}", name=f"outT{k}") for k in range(2)]

            # staging tiles (persistent; zero rows written once)
            ks8 = [cp.tile([128, T], FP8, tag=f"ks8_{i}", name=f"ks8_{i}") for i in range(2)]
            qs8 = [cp.tile([128, 512], FP8, tag=f"qs8_{i}", name=f"qs8_{i}") for i in range(6)]
            ksb = [cp.tile([128, 512], BF16, tag=f"ksb_{i}", name=f"ksb_{i}") for i in range(2)]
            qsb = [cp.tile([128, 512], BF16, tag=f"qsb_{i}", name=f"qsb_{i}") for i in range(2)]

            xt8_r = xt8[:].rearrange("p (k n) -> p k n", k=KC)

            # ---- input DMA: interleave wqk8/xt8 pairs (stage B critical path),
            # then everything else ----
            nc.sync.dma_start(wqk8[:], wqk8_d[:])
            for kp in range(KC // 2):
                nc.sync.dma_start(
                    xt8[:].rearrange("p (k n) -> p k n", k=KC)[:, 2 * kp:2 * kp + 2, :],
                    xt8_d.rearrange("(k p) n -> p k n", k=KC)[:, 2 * kp:2 * kp + 2, :])
            for m in range(4):
                nc.sync.dma_start(bqk64[m][:], bqk64_d[128 * m:128 * (m + 1), :])
                nc.sync.dma_start(bqkt[m][:], bqkt_d[128 * m:128 * (m + 1), :])
            nc.sync.dma_start(wvb[:].rearrange("p (k n) -> p k n", k=KC),
                              wvb_d.rearrange("(k p) n -> p k n", k=KC))
            nc.sync.dma_start(bvb[:], bvb_d[:])
            for half in range(2):
                nc.sync.dma_start(
                    xtb[:].rearrange("p (k n) -> p k n", k=KC)[:, 4 * half:4 * half + 4, :],
                    xtb_d.rearrange("(k p) n -> p k n", k=KC)[:, 4 * half:4 * half + 4, :])
            nc.sync.dma_start(maskA[:], maskA_d[:])
            nc.sync.dma_start(maskB[:], maskB_d[:])
            nc.sync.dma_start(trib[:], trib_d[:])
            for k in range(2):
                nc.sync.dma_start(wpb[k][:], wpb_d[128 * k:128 * (k + 1), :])
            nc.vector.memset(onesb[:], 1.0)
            # zero the pad rows of the staging tiles once
            for t in ks8:
                nc.gpsimd.memset(t[64:128, :], 0)
            for t in qs8:
                nc.gpsimd.memset(t[64:128, :], 0)
            for t in ksb:
                nc.gpsimd.memset(t[64:128, :], 0)
            for t in qsb:
                nc.gpsimd.memset(t[64:128, :], 0)

            # ---- stage B: q/k projection, fp8 DoubleRow ----
            # out channels: [q(0:256), k(256:512)]; mf chunks of 128.
            # group A = (mf0, mf2) -> heads 0,1 done first.
            with tc.tile_pool(name="psB", bufs=1, space="PSUM") as psB:
                for gi, mf in enumerate([0, 2, 1, 3]):
                    pss = [psB.tile([128, 512], F32, tag=f"psB{gi % 2}_{nt}",
                                    name=f"psB{mf}_{nt}") for nt in range(NT)]
                    for kp in range(KC // 2):
                        for nt in range(NT):
                            wb = (kp * 8 + mf * 2) * 128
                            nc.tensor.matmul(
                                pss[nt][:],
                                wqk8[:, wb:wb + 256].rearrange(
                                    "p (two m) -> p two m", two=2),
                                xt8_r[:, 2 * kp:2 * kp + 2,
                                      512 * nt:512 * (nt + 1)],
                                start=(kp == 0), stop=(kp == KC // 2 - 1),
                                perf_mode=DoubleRow)
                    for nt in range(NT):
                        ps = pss[nt]
                        # fp8 q'/k' at 64x scale (+64x bias); ACT evacuates
                        nc.scalar.activation(
                            qk8[mf][:, 512 * nt:512 * (nt + 1)], ps[:],
                            Identity, bias=bqk64[mf][:], scale=1.0)
                        # bf16 true-scale for the j0 path (DVE, off ACT)
                        if mf in (0, 1) and nt == 0:
                            nc.vector.tensor_scalar(
                                qkb[mf][:], ps[:], 1.0 / WSCALE, bqkt[mf][:],
                                op0=MULT, op1=ADD)
                        if mf in (2, 3) and nt == 0:
                            nc.vector.tensor_scalar(
                                kkb[mf - 2][:], ps[:], 1.0 / WSCALE, bqkt[mf][:],
                                op0=MULT, op1=ADD)

            # ---- stage C: v projection, bf16 (+ ones col via K=1 bias matmul) ----
            with tc.tile_pool(name="psC", bufs=3, space="PSUM") as psC:
                for mt in range(T // 128):
                    ps = psC.tile([128, VW], F32, tag="psv", name=f"psv{mt}")
                    for kk in range(KC):
                        nc.tensor.matmul(
                            ps[:],
                            xtb[:, T * kk + 128 * mt:T * kk + 128 * (mt + 1)],
                            wvb[:, VW * kk:VW * (kk + 1)],
                            start=(kk == 0), stop=False)
                    nc.tensor.matmul(ps[:], onesb[:, :], bvb[:],
                                     start=False, stop=True)
                    nc.scalar.activation(v8[:, VW * mt:VW * (mt + 1)],
                                             ps[:], Copy)
                    if mt < 4:
                        nc.vector.tensor_copy(vb[:, VW * mt:VW * (mt + 1)],
                                              ps[:])
                    if mt % 2 == 1:
                        pp = mt // 2
                        for hh in range(HPG):
                            for s in range(2):
                                dst = ((pp * 4 + hh) * 2 + s) * 128
                                nc.sync.dma_start(
                                    v8p[:, dst:dst + 128],
                                    v8[:, VW * (2 * pp + s) + 65 * hh:
                                       VW * (2 * pp + s) + 65 * hh + 128])

            # ---- stage D: attention ----
            # pair schedule per head: j0 (bf16) pairs then j>=1 (fp8 DR) pairs
            with (
                tc.tile_pool(name="psA", bufs=4, space="PSUM") as psA,
                tc.tile_pool(name="psAV", bufs=1, space="PSUM") as psAV,
                tc.tile_pool(name="etp", bufs=6) as etp,
                tc.tile_pool(name="et0p", bufs=3) as et0p,
                tc.tile_pool(name="recp", bufs=2) as recp,
                tc.tile_pool(name="bcp", bufs=2) as bcp,
            ):
                act_load = [0.0]   # running col-count per engine for balance
                dve_load = [0.0]

                for h in range(HPG):
                    qtile, off = h // 2, 64 * (h % 2)
                    ktile = 2 + h // 2
                    # --- staging DMAs ---
                    ks = ks8[h % 2]
                    nc.sync.dma_start(ks[0:64, :], qk8[ktile][off:off + 64, :])
                    kb = ksb[h % 2]
                    nc.sync.dma_start(kb[0:64, :], kkb[h // 2][off:off + 64, 0:512])
                    qb = qsb[h % 2]
                    nc.sync.dma_start(qb[0:64, :], qkb[h // 2][off:off + 64, :])
                    qss = {}
                    for j in range(1, NT):
                        t = qs8[(h % 2) * 3 + (j - 1)]
                        nc.sync.dma_start(
                            t[0:64, :],
                            qk8[qtile][off:off + 64, 512 * j:512 * (j + 1)])
                        qss[j] = t

                    avp = {}
                    dve_load[0] += 3500.0   # norm/recip burden this head
                    act_load[0] += 1300.0

                    # pair list: (j, p, c0pair, kind)
                    pairs = []
                    pairs.append((0, 0, 0, "j0"))
                    pairs.append((0, 1, 256, "j0"))
                    for j in range(1, NT):
                        for p in range(2 * j + 2):
                            i0 = 2 * p
                            if i0 == 4 * j:
                                pairs.append((j, p, 0, "dA"))
                            elif i0 == 4 * j + 2:
                                pairs.append((j, p, 256, "dB"))
                            else:
                                pairs.append((j, p, 0, "nd"))

                    pending_av = []

                    def emit_av(rec):
                        j, p, c0p, kind, et = rec
                        if p == 0:
                            avp[j] = psAV.tile([128, 512], F32, tag=f"avj{j}",
                                               name=f"avps{h}_{j}")
                        if kind == "j0":
                            for s in range(2):
                                i = 2 * p + s
                                cc = 128 * i
                                nc.tensor.matmul(
                                    avp[0][:, cc:512],
                                    vb[:, VW * i + 65 * h:VW * i + 65 * h + 128],
                                    et[:, 512 * s + cc:512 * s + 512],
                                    start=(i == 0), stop=(i == 3))
                            if p == 1:
                                normalize(h, 0)
                        else:
                            if c0p == 0:
                                et_r = et[:].bitcast(FP8).rearrange(
                                    "p (two n) -> p two n", two=2)
                            else:
                                et_r = et[:].bitcast(FP8)[
                                    :, 256:768].rearrange(
                                    "p (two n) -> p two n", two=2)
                            vbase = ((p * 4 + h) * 2) * 128
                            nc.tensor.matmul(
                                avp[j][:, c0p:512],
                                v8p[:, vbase:vbase + 256].rearrange(
                                    "p (two m) -> p two m", two=2),
                                et_r,
                                start=(p == 0), stop=(p == 2 * j + 1),
                                perf_mode=DoubleRow)
                            if p == 2 * j + 1:
                                normalize(h, j)

                    def normalize(h_, j_):
                        u = h_ * NT + j_
                        rowb = recp.tile([1, 512], F32, tag="rowb",
                                         name=f"rowb{h_}_{j_}")
                        if (h_ + j_) % 2 == 0:
                            nc.scalar.activation(rowb[:], avp[j_][64:65, :],
                                                 Copy)
                        else:
                            nc.vector.tensor_copy(rowb[:], avp[j_][64:65, :])
                        nc.sync.dma_start(den_d[u:u + 1, :], rowb[:])
                        den2 = recp.tile([128, 4], F32, tag="den2",
                                         name=f"den2_{h_}_{j_}")
                        nc.sync.dma_start(
                            den2[:], bass.AP(den_d.tensor, u * 512,
                                             [[4, 128], [1, 4]]))
                        rec2 = recp.tile([128, 4], F32, tag="rec2",
                                         name=f"rec2_{h_}_{j_}")
                        nc.vector.reciprocal(rec2[:], den2[:])
                        nc.sync.dma_start(
                            bass.AP(rec_d.tensor, u * 512, [[4, 128], [1, 4]]),
                            rec2[:])
                        bc = bcp.tile([64, 512], F32, tag="bc",
                                      name=f"bc{h_}_{j_}")
                        nc.sync.dma_start(
                            bc[:], bass.AP(rec_d.tensor, u * 512,
                                           [[0, 64], [1, 512]]))
                        off_ = 64 * (h_ % 2)
                        nc.vector.scalar_tensor_tensor(
                            outT[h_ // 2][off_:off_ + 64,
                                          512 * j_:512 * (j_ + 1)],
                            avp[j_][0:64, :], 1.0, bc[:],
                            op0=MULT, op1=MULT)

                    LAG = 4
                    for ui, (j, p, c0p, kind) in enumerate(pairs):
                        # attT: per-UNIT 1-bank psum tiles so unit u+4's
                        # alloc waits only exp(u), not the whole pair
                        if kind == "j0":
                            et = et0p.tile([128, 1024], BF16, tag="et0",
                                           name=f"et0_{h}_{p}")
                            for s in range(2):
                                i = 2 * p + s
                                cc = 128 * i
                                aps = psA.tile([128, 512], F32, tag="aps",
                                               name=f"aps{h}_{j}_{p}_{s}")
                                nc.tensor.matmul(
                                    aps[:, cc:512],
                                    kb[:, 128 * i:128 * (i + 1)],
                                    qb[:, cc:512], start=True, stop=True)
                                nc.scalar.activation(
                                    et[:, 512 * s + cc:512 * s + 512],
                                    aps[:, cc:512],
                                    Exp, scale=0.125)
                                act_load[0] += 512 - cc
                                # mask diag block via Pool multiply
                                nc.gpsimd.tensor_mul(
                                    et[:, 512 * s + cc:512 * s + cc + 128],
                                    et[:, 512 * s + cc:512 * s + cc + 128],
                                    trib[:])
                        else:
                            et = etp.tile([128, 1024], I8, tag="et",
                                          name=f"et_{h}_{j}_{p}")
                            for s in range(2):
                                i = 2 * p + s
                                c0u = c0p if kind == "nd" else (
                                    128 * ((2 * p + s) % 4))
                                aps = psA.tile([128, 512], F32, tag="aps",
                                               name=f"aps{h}_{j}_{p}_{s}")
                                nc.tensor.matmul(
                                    aps[:, c0u:512],
                                    ks[:, 128 * i:128 * (i + 1)],
                                    qss[j][:, c0u:512], start=True, stop=True)
                                # exp right after this half's attT
                                if kind == "nd":
                                    if act_load[0] <= dve_load[0]:
                                        nc.scalar.activation(
                                            et[:].bitcast(FP8)[
                                                :, 512 * s:512 * (s + 1)],
                                            aps[:],
                                            Exp, scale=A_BYTE * LN2 / 8.0)
                                        act_load[0] += 512
                                    else:
                                        nc.vector.tensor_scalar(
                                            et[:, 512 * s:512 * (s + 1)],
                                            aps[:],
                                            A_BYTE, B_BYTE,
                                            op0=MULT, op1=ADD)
                                        dve_load[0] += 512
                                elif kind == "dA":
                                    nc.vector.scalar_tensor_tensor(
                                        et[:, 512 * s:512 * (s + 1)],
                                        aps[:],
                                        A_BYTE,
                                        maskA[:, 512 * s:512 * (s + 1)],
                                        op0=MULT, op1=ADD)
                                    dve_load[0] += 512
                                else:  # dB: valid psum cols [256:512); et
                                    # compacted to [256:512)+[512:768)
                                    nc.vector.scalar_tensor_tensor(
                                        et[:, 256 * (s + 1):256 * (s + 2)],
                                        aps[:, 256:512],
                                        A_BYTE,
                                        maskB[:, 256 * s:256 * (s + 1)],
                                        op0=MULT, op1=ADD)
                                    dve_load[0] += 256
                        pending_av.append((j, p, c0p, kind, et))
                        if len(pending_av) > LAG:
                            emit_av(pending_av.pop(0))
                    while pending_av:
                        emit_av(pending_av.pop(0))

            # ---- stage E: y^T partial = wp.T @ outT, bf16 ----
            with (
                tc.tile_pool(name="psP", bufs=2, space="PSUM") as psP,
                tc.tile_pool(name="outp", bufs=4) as outp,
            ):
                for mo in range(8):
                    pss = [psP.tile([128, 512], F32, tag=f"psP{nt}",
                                    name=f"psP{mo}_{nt}") for nt in range(NT)]
                    for k in range(2):
                        for nt in range(NT):
                            nc.tensor.matmul(
                                pss[nt][:], wpb[k][:, 128 * mo:128 * (mo + 1)],
                                outT[k][:, 512 * nt:512 * (nt + 1)],
                                start=(k == 0), stop=(k == 1))
                    ot = outp.tile([128, T], BF16, tag="ot",
                                   name=f"ot{mo}")
                    for nt in range(NT):
                        if nt % 2 == 0:
                            nc.vector.tensor_copy(
                                ot[:, 512 * nt:512 * (nt + 1)], pss[nt][:])
                        else:
                            nc.scalar.activation(
                                ot[:, 512 * nt:512 * (nt + 1)], pss[nt][:],
                                Copy)
                    nc.sync.dma_start(yt_d[128 * mo:128 * (mo + 1), :], ot[:])

    nc.compile()
    return nc


def _shard_inputs(x, w_qkv, b_qkv, w_proj):
    e4 = ml_dtypes.float8_e4m3fn
    bf = ml_dtypes.bfloat16
    in_maps = []
    r = np.arange(128)[:, None]
    c = np.arange(128)[None, :]
    tri01 = (c >= r).astype(np.float32)            # keep iff q >= k
    # byte-domain additive masks: keep -> 56.0, masked -> -1e4
    def seg_tri(q0):
        # [128, 128] tri: rows k-local, cols q-local, keep iff (q0+c) >= r
        return np.where(c >= r, B_BYTE, MASKV).astype(np.float32)
    keep = np.full((128, 128), B_BYTE, np.float32)
    full = np.full((128, 128), MASKV, np.float32)
    maskA = np.concatenate(
        [seg_tri(0), keep, keep, keep,            # slot0: tri@0, open
         full, seg_tri(0), keep, keep], axis=1)   # slot1: full@0, tri@128, open
    maskB = np.concatenate(
        [seg_tri(0), keep,                        # slot0 (cols 256-511): tri, open
         full, seg_tri(0)], axis=1)               # slot1: full, tri
    trib = tri01.astype(bf)

    for core in range(N_CORES):
        b, g = divmod(core, HPG)
        qs = slice(CG * g, CG * (g + 1))
        ks = slice(C + CG * g, C + CG * (g + 1))
        vs = slice(2 * C + CG * g, 2 * C + CG * (g + 1))
        # wqk8: [C, 512] = 64 * [Wq; Wk]^T in e4m3 (no attention scale folded;
        # it lives in A_BYTE / the j0 exp scale)
        wqk = np.concatenate([w_qkv[qs] * WSCALE, w_qkv[ks] * WSCALE],
                             axis=0).T
        # paired dual-fp8 layout: [p, kp, mf, s, m] -> [128, 4096]
        wqk = np.ascontiguousarray(
            wqk.reshape(4, 2, 128, 4, 128).transpose(2, 0, 3, 1, 4)
            .reshape(128, 4096))
        bqk64 = (np.concatenate([b_qkv[qs], b_qkv[ks]]) * WSCALE)[:, None]
        bqkt = np.concatenate([b_qkv[qs], b_qkv[ks]])[:, None]
        wv_base = w_qkv[vs].T
        wv = np.zeros((C, VW), np.float32)
        bv = np.zeros((1, VW), np.float32)
        for h in range(HPG):
            wv[:, 65 * h:65 * h + 64] = wv_base[:, 64 * h:64 * h + 64]
            bv[0, 65 * h:65 * h + 64] = b_qkv[vs][64 * h:64 * h + 64]
            bv[0, 65 * h + 64] = 1.0
        xt = np.ascontiguousarray(x[b].T, np.float32)
        in_maps.append({
            "xt8": xt.astype(e4),
            "xtb": xt.astype(bf),
            "wqk8": np.ascontiguousarray(wqk).astype(e4),
            "bqk64": np.ascontiguousarray(bqk64, np.float32),
            "bqkt": np.ascontiguousarray(bqkt, np.float32),
            "wvb": wv.astype(bf),
            "bvb": bv.astype(bf),
            "wpb": np.ascontiguousarray(
                w_proj[:, CG * g:CG * (g + 1)].T).astype(bf),
            "maskA": maskA,
            "maskB": maskB,
            "trib": trib,
        })
    return in_maps


def kernel(x, w_qkv, b_qkv, w_proj, b_proj):
    x = np.asarray(x, np.float32)
    w_qkv = np.asarray(w_qkv, np.float32)
    b_qkv = np.asarray(b_qkv, np.float32)
    w_proj = np.asarray(w_proj, np.float32)
    b_proj = np.asarray(b_proj, np.float32)

    nc = build_nc()
    in_maps = _shard_inputs(x, w_qkv, b_qkv, w_proj)
    if _trace_flag[0]:
        _ensure_ntff_hook()
    res = run_bass_kernel_spmd(nc, in_maps, core_ids=list(range(N_CORES)),
                               trace=_trace_flag[0])
    _last_results[0] = res

    y = np.empty((B, T, C), np.float32)
    for b in range(B):
        acc = np.zeros((C, T), np.float32)
        for g in range(HPG):
            acc += np.asarray(res.results[HPG * b + g]["yt"], np.float32)
        y[b] = acc.T + b_proj[None, :]
    return y
